# revision 1
# baseline (speedup 1.0000x reference)
"""Distance-aware multihead attention on 8 Trainium2 NeuronCores.

Problem: B=4, S=1024, D=768, H=12, DK=64, NUM_EMB=10.
  q/k/v = linear projections of query/key/value
  idx[b,i,j] = clip(round(9 * |pos_i - pos_j| / MAXD), 0, 9)
  logits = (q.k^T + qe[b,h,i,idx[b,i,j]]) / 8   where qe = q @ emb_k^T
  out = softmax(logits) @ v

Key decompositions:
  - bias qe[...,idx] = qe[...,0] + sum_{e=1..9} (qe_e - qe_{e-1}) * (d2 >= T_e^2);
    the qe_0 term is constant along the softmax axis and cancels -> dropped.
  - step masks (d2 >= T_e^2) are shared across all 12 heads of a q-tile.
  - bias accumulated onto QK logits via 9 scalar_tensor_tensor ops per (head, q-tile).

Sharding: core c handles batch c//2, query-half c%2 (512 queries, all heads).
K/V/projections are computed per-core from full-S inputs (duplicated across the
2 cores sharing a batch); masks/logits/AV are not duplicated.

Layouts: Q^T/K^T [dim, token] f32r (from projections), V [token, dim] bf16.
P = exp((qk+bias)/8) bf16 in [q, k]; transposed to [k, q] 128-chunks via the
DMA-xbar transpose engine; AV accumulates over the 8 k-chunks on TensorE.
"""
import os
import numpy as np

import concourse.bass as bass
import concourse.tile as tile
from concourse import bacc, mybir
from concourse.bass_utils import run_bass_kernel_spmd

F32 = mybir.dt.float32
F32R = mybir.dt.float32r
BF16 = mybir.dt.bfloat16
ACT = mybir.ActivationFunctionType
ALU = mybir.AluOpType

B, S, D = 4, 1024, 768
H, DK = 12, 64
NUM_EMB = 10
MAX_DIST = 100000.0 * 2 ** 0.5
SQ = S // 2          # queries per core
NQT = SQ // 128      # q-tiles per core (4)
NKT = S // 128       # k token chunks (8)
NDT = D // 128       # dim tiles (6)
NCORES = 8

# squared thresholds: idx >= e  <=>  d2 >= ((e-0.5)*MAX_DIST/9)^2
THRESH2 = [float(((e - 0.5) * MAX_DIST / 9.0) ** 2) for e in range(1, NUM_EMB)]


def _load_T(nc, dst, src_dram, ncols):
    """src [rows, ncols*64] DRAM -> dst [128, ncols_grp, rows] = src^T, via
    64-partition xbar transpose chunks. dst is [128, n, rows] with
    dst[(64j)%128 + p64, j//2, r] = src[r, 64j + p64]."""
    for j in range(ncols // 64):
        nc.sync.dma_start_transpose(
            dst[(64 * j) % 128:(64 * j) % 128 + 64, j // 2, :],
            src_dram[:, 64 * j:64 * j + 64])


def build_nc(stage="full"):
    nc = bacc.Bacc("TRN2", target_bir_lowering=False, debug=False)

    # matmul-feeding inputs are float32r so the fp32r verifier accepts
    # DMA -> SBUF -> matmul (host values are plain fp32 bits).
    xq = nc.dram_tensor("xq", [SQ, D], F32R, kind="ExternalInput").ap()
    xk = nc.dram_tensor("xk", [S, D], F32R, kind="ExternalInput").ap()
    xv = nc.dram_tensor("xv", [S, D], F32R, kind="ExternalInput").ap()
    pos = nc.dram_tensor("pos", [S, 2], F32, kind="ExternalInput").ap()
    posq = nc.dram_tensor("posq", [SQ, 2], F32, kind="ExternalInput").ap()
    wq = nc.dram_tensor("wq", [D, D], F32R, kind="ExternalInput").ap()
    wk = nc.dram_tensor("wk", [D, D], F32R, kind="ExternalInput").ap()
    wv = nc.dram_tensor("wv", [D, D], F32R, kind="ExternalInput").ap()
    bq = nc.dram_tensor("bq", [D], F32, kind="ExternalInput").ap()
    bk = nc.dram_tensor("bk", [D], F32, kind="ExternalInput").ap()
    bv = nc.dram_tensor("bv", [D], F32, kind="ExternalInput").ap()
    emb = nc.dram_tensor("emb", [NUM_EMB, DK], F32R, kind="ExternalInput").ap()
    out = nc.dram_tensor("out", [SQ, D], F32, kind="ExternalOutput").ap()

    # debug stages: "proj" stops after projections, "masks" after d2/masks,
    # "logits" skips transpose+AV, "notrans" replaces the P transpose with a
    # plain DMA (wrong values, isolates the xbar), "full" is the real kernel.
    with tile.TileContext(nc) as tc:
        with tc.tile_pool(name="persist", bufs=1) as persist:
            # ---- setup: bias columns, position broadcasts ----
            bq_col = persist.tile([128, NDT], F32)
            bk_col = persist.tile([128, NDT], F32)
            nc.sync.dma_start(out=bq_col[:], in_=bass.AP(tensor=bq.tensor, offset=0, ap=[[1, 128], [128, NDT]]))
            nc.sync.dma_start(out=bk_col[:], in_=bass.AP(tensor=bk.tensor, offset=0, ap=[[1, 128], [128, NDT]]))
            bv_b = persist.tile([128, D], F32)
            nc.sync.dma_start(out=bv_b[:], in_=bass.AP(tensor=bv.tensor, offset=0, ap=[[0, 128], [1, D]]))
            xk_b = persist.tile([128, S], F32)
            yk_b = persist.tile([128, S], F32)
            nc.sync.dma_start(out=xk_b[:], in_=bass.AP(tensor=pos.tensor, offset=0, ap=[[0, 128], [2, S]]))
            nc.sync.dma_start(out=yk_b[:], in_=bass.AP(tensor=pos.tensor, offset=1, ap=[[0, 128], [2, S]]))
            # query positions as per-partition scalars [128, NQT]
            xq_col = persist.tile([128, NQT], F32)
            yq_col = persist.tile([128, NQT], F32)
            nc.sync.dma_start(out=xq_col[:], in_=bass.AP(tensor=posq.tensor, offset=0, ap=[[2, 128], [256, NQT]]))
            nc.sync.dma_start(out=yq_col[:], in_=bass.AP(tensor=posq.tensor, offset=1, ap=[[2, 128], [256, NQT]]))
            # emb^T on both 64-partition halves
            embT = persist.tile([128, NUM_EMB], F32R)
            nc.sync.dma_start_transpose(embT[0:64, :], emb[:, :])
            nc.sync.dma_start_transpose(embT[64:128, :], emb[:, :])
            embT_blk = persist.tile([128, 2 * NUM_EMB], F32R)
            nc.vector.memset(embT_blk[:].bitcast(F32), 0.0)
            nc.sync.dma_start_transpose(embT_blk[0:64, 0:NUM_EMB], emb[:, :])
            nc.sync.dma_start_transpose(embT_blk[64:128, NUM_EMB:2 * NUM_EMB], emb[:, :])

            ident = persist.tile([128, 128], BF16)
            from concourse.masks import make_identity
            make_identity(nc, ident[:])
            v_sb = persist.tile([128, NKT, D], BF16)   # V[token, dim], token-chunked
            kT = persist.tile([128, NDT, S], F32R)     # K^T[dim, token]
            qT = persist.tile([128, NDT, SQ], F32R)    # Q^T[dim, token]

            # ---- projections (phased so X^T/W^T buffers are freed early) ----
            with tc.tile_pool(name="vproj", bufs=1) as vp, \
                 tc.tile_pool(name="vps", bufs=2, space="PSUM") as vps:
                wvT = vp.tile([128, NDT, D], F32R)
                xvT = vp.tile([128, NDT, S], F32R)
                _load_T(nc, wvT, wv, D)
                _load_T(nc, xvT, xv, D)
                for m in range(NKT):
                    for hf in range(2):
                        ps = vps.tile([128, 384], F32, tag="pj")
                        for t in range(NDT):
                            nc.tensor.matmul(ps[:], xvT[:, t, 128 * m:128 * m + 128],
                                             wvT[:, t, 384 * hf:384 * hf + 384],
                                             start=(t == 0), stop=(t == NDT - 1))
                        nc.scalar.copy(v_sb[:, m, 384 * hf:384 * hf + 384], ps[:])

            with tc.tile_pool(name="kproj", bufs=1) as kp, \
                 tc.tile_pool(name="kps", bufs=2, space="PSUM") as kps:
                wkT = kp.tile([128, NDT, D], F32R)
                xkT = kp.tile([128, NDT, S], F32R)
                _load_T(nc, wkT, wk, D)
                _load_T(nc, xkT, xk, D)
                for m in range(NDT):
                    for hf in range(2):
                        ps = kps.tile([128, 512], F32, tag="pj")
                        for t in range(NDT):
                            nc.tensor.matmul(ps[:], wkT[:, t, 128 * m:128 * m + 128],
                                             xkT[:, t, 512 * hf:512 * hf + 512],
                                             start=(t == 0), stop=(t == NDT - 1))
                        nc.scalar.activation(kT[:, m, 512 * hf:512 * hf + 512], ps[:],
                                             ACT.Identity, bias=bk_col[:, m:m + 1])

            with tc.tile_pool(name="qproj", bufs=1) as qp, \
                 tc.tile_pool(name="qps", bufs=2, space="PSUM") as qps:
                wqT = qp.tile([128, NDT, D], F32R)
                xqT = qp.tile([128, NDT, SQ], F32R)
                _load_T(nc, wqT, wq, D)
                _load_T(nc, xqT, xq, D)
                for m in range(NDT):
                    ps = qps.tile([128, 512], F32, tag="pj")
                    for t in range(NDT):
                        nc.tensor.matmul(ps[:], wqT[:, t, 128 * m:128 * m + 128],
                                         xqT[:, t, :],
                                         start=(t == 0), stop=(t == NDT - 1))
                    nc.scalar.activation(qT[:, m, :], ps[:], ACT.Identity,
                                         bias=bq_col[:, m:m + 1])

            if stage == "proj":
                # dump some projection results and stop
                with tc.tile_pool(name="dump", bufs=1) as dp:
                    t = dp.tile([128, 512], F32)
                    nc.scalar.copy(t[:], qT[:, 0, :].bitcast(F32))
                    nc.sync.dma_start(out=out[0:128, 0:512], in_=t[:])
                    t2 = dp.tile([128, 512], F32)
                    nc.scalar.copy(t2[:], kT[:, 0, 0:512].bitcast(F32))
                    nc.sync.dma_start(out=out[128:256, 0:512], in_=t2[:])
                    t3 = dp.tile([128, 512], F32)
                    nc.vector.tensor_copy(t3[:], v_sb[:, 0, 0:512])
                    nc.sync.dma_start(out=out[256:384, 0:512], in_=t3[:])

            # ---- attention ----
            if os.environ.get("BARRIER"):
                tc.strict_bb_all_engine_barrier()
            if not os.environ.get("NOWARMXP"):
                # dummy 2-byte xbar transpose: the first 2B transpose after the
                # 4B setup transposes produces garbage (xbar mode transition);
                # this one absorbs it.
                scrap = persist.tile([128, 128], BF16)
                scrapT = persist.tile([128, 128], BF16)
                nc.vector.memset(scrap[:], 0.0)
                nc.sync.dma_start_transpose(scrapT[:], scrap[:])
            if stage != "proj":
              with tc.tile_pool(name="att", bufs=2) as att, \
                 tc.tile_pool(name="accp", bufs=2) as accp, \
                 tc.tile_pool(name="qe_ps", bufs=1, space="PSUM") as qe_ps, \
                 tc.tile_pool(name="qk_ps", bufs=2, space="PSUM") as qk_ps, \
                 tc.tile_pool(name="pt_ps", bufs=1, space="PSUM") as pt_ps, \
                 tc.tile_pool(name="av_ps", bufs=2, space="PSUM") as av_ps:
                for qt in range(1 if os.environ.get("NQT1") else (NQT if (stage not in ("masks", "logits", "d2") or os.environ.get("FULLLOOPS")) else 1)):
                    if os.environ.get("QTBARRIER"):
                        tc.strict_bb_all_engine_barrier()
                    if os.environ.get("NOMASKS"):
                        masks = att.tile([128, NUM_EMB - 1, S], BF16, tag="masks")
                        dqe = att.tile([128, H, NUM_EMB - 1], F32, tag="dqe")
                        if os.environ.get("DOD2"):
                            dx = att.tile([128, S], F32, tag="dx")
                            dy = att.tile([128, S], F32, tag="dy")
                            nc.vector.tensor_scalar(out=dx[:], in0=xk_b[:], scalar1=xq_col[:, qt:qt + 1],
                                                    scalar2=None, op0=ALU.subtract)
                            nc.vector.tensor_scalar(out=dy[:], in0=yk_b[:], scalar1=yq_col[:, qt:qt + 1],
                                                    scalar2=None, op0=ALU.subtract)
                            dx2 = att.tile([128, S], F32, tag="dx2")
                            dy2 = att.tile([128, S], F32, tag="dy2")
                            nc.scalar.square(dx2[:], dx[:])
                            nc.scalar.square(dy2[:], dy[:])
                            d2 = att.tile([128, S], F32, tag="d2")
                            nc.vector.tensor_add(d2[:], dx2[:], dy2[:])
                            if os.environ.get("DOMASKS"):
                                for e in range(NUM_EMB - 1):
                                    nc.vector.tensor_scalar(out=masks[:, e, :], in0=d2[:],
                                                            scalar1=THRESH2[e], scalar2=None,
                                                            op0=ALU.is_ge)
                        if os.environ.get("SECTBARRIER"):
                            tc.strict_bb_all_engine_barrier()
                        if os.environ.get("DOQE"):
                            qe_psum = qe_ps.tile([128, H * NUM_EMB], F32, tag="qe")
                            if os.environ.get("QEBLK"):
                                for m in range(NDT):
                                    nc.tensor.matmul(qe_psum[:, 20 * m:20 * m + 20],
                                                     qT[:, m, 128 * qt:128 * qt + 128],
                                                     embT_blk[:],
                                                     start=True, stop=True)
                            else:
                                for h in range(H):
                                    off = (64 * h) % 128
                                    nc.tensor.matmul(qe_psum[:, 10 * h:10 * h + 10],
                                                     qT[off:off + 64, h // 2, 128 * qt:128 * qt + 128],
                                                     embT[off:off + 64, :],
                                                     start=True, stop=True)
                            qe_sb = att.tile([128, H, NUM_EMB], F32, tag="qe_sb")
                            nc.scalar.copy(qe_sb[:], qe_psum[:].rearrange("p (h e) -> p h e", e=NUM_EMB))
                            nc.vector.tensor_tensor(out=dqe[:], in0=qe_sb[:, :, 1:],
                                                    in1=qe_sb[:, :, :-1], op=ALU.subtract)
                        if os.environ.get("SECTBARRIER"):
                            tc.strict_bb_all_engine_barrier()
                        for h in range(H):
                            off = 0 if os.environ.get("OFF0") else (64 * h) % 128
                            qk = qk_ps.tile([128, S], F32, tag="qk")
                            for hf in range(2):
                                nc.tensor.matmul(qk[:, 512 * hf:512 * hf + 512],
                                                 qT[off:off + 64, h // 2, 128 * qt:128 * qt + 128],
                                                 kT[off:off + 64, h // 2, 512 * hf:512 * hf + 512],
                                                 start=True, stop=True)
                            o3 = att.tile([128, DK], F32, tag="o")
                            nc.scalar.copy(o3[:], qk[:, 0:DK])
                            nc.sync.dma_start(out=out[128 * qt:128 * qt + 128, 64 * h:64 * h + 64],
                                              in_=o3[:])
                        continue
                    # --- d2 for this q-tile: [128, S] fp32 ---
                    dx = att.tile([128, S], F32, tag="dx")
                    dy = att.tile([128, S], F32, tag="dy")
                    nc.vector.tensor_scalar(out=dx[:], in0=xk_b[:], scalar1=xq_col[:, qt:qt + 1],
                                            scalar2=None, op0=ALU.subtract)
                    nc.vector.tensor_scalar(out=dy[:], in0=yk_b[:], scalar1=yq_col[:, qt:qt + 1],
                                            scalar2=None, op0=ALU.subtract)
                    dx2 = att.tile([128, S], F32, tag="dx2")
                    dy2 = att.tile([128, S], F32, tag="dy2")
                    nc.scalar.square(dx2[:], dx[:])
                    nc.scalar.square(dy2[:], dy[:])
                    d2 = att.tile([128, S], F32, tag="d2")
                    nc.vector.tensor_add(d2[:], dx2[:], dy2[:])

                    if stage == "qeonly":
                        qe_psum = qe_ps.tile([128, H * NUM_EMB], F32, tag="qe")
                        for h in range(H):
                            off = (64 * h) % 128
                            nc.tensor.matmul(qe_psum[:, 10 * h:10 * h + 10],
                                             qT[off:off + 64, h // 2, 128 * qt:128 * qt + 128],
                                             embT[off:off + 64, :],
                                             start=True, stop=True)
                        qe_sb = att.tile([128, H, NUM_EMB], F32, tag="qe_sb")
                        nc.scalar.copy(qe_sb[:], qe_psum[:].rearrange("p (h e) -> p h e", e=NUM_EMB))
                        dqe = att.tile([128, H, NUM_EMB - 1], F32, tag="dqe")
                        nc.vector.tensor_tensor(out=dqe[:], in0=qe_sb[:, :, 1:],
                                                in1=qe_sb[:, :, :-1], op=ALU.subtract)
                        o4 = att.tile([128, DK], F32, tag="o")
                        nc.vector.tensor_copy(o4[:, 0:63], dqe[:, 0:7, 0:9].rearrange("p a b -> p (a b)"))
                        nc.vector.tensor_copy(o4[:, 63:64], dqe[:, 7, 0:1])
                        nc.sync.dma_start(out=out[128 * qt:128 * qt + 128, 0:DK], in_=o4[:])
                        continue

                    if stage == "d2":
                        nc.sync.dma_start(out=out[128:256, 0:D], in_=d2[:, 0:D])
                        continue

                    # --- step masks [128, 9, S] bf16 ---
                    nmask = int(os.environ.get("NMASKS", str(NUM_EMB - 1)))
                    mdt = F32 if os.environ.get("MASKF32") else BF16
                    masks = att.tile([128, NUM_EMB - 1, S], mdt, tag="masks")
                    for e in range(nmask):
                        if os.environ.get("MASKCOPY"):
                            nc.vector.tensor_copy(masks[:, e, :], d2[:])
                        elif os.environ.get("MASKIMM1"):
                            nc.vector.tensor_scalar(out=masks[:, e, :], in0=d2[:],
                                                    scalar1=1.0, scalar2=None,
                                                    op0=ALU.is_ge)
                        else:
                            nc.vector.tensor_scalar(out=masks[:, e, :], in0=d2[:],
                                                    scalar1=THRESH2[e], scalar2=None,
                                                    op0=ALU.is_ge)

                    # --- qe -> dqe for this q-tile (block-diagonal: 2 heads per matmul;
                    # 64-partition sliver matmuls into one bank proved flaky on HW) ---
                    qe_psum = qe_ps.tile([128, H * NUM_EMB], F32, tag="qe")
                    for m in range(NDT):
                        nc.tensor.matmul(qe_psum[:, 20 * m:20 * m + 20],
                                         qT[:, m, 128 * qt:128 * qt + 128],
                                         embT_blk[:],
                                         start=True, stop=True)
                    qe_sb = att.tile([128, H, NUM_EMB], F32, tag="qe_sb")
                    nc.scalar.copy(qe_sb[:], qe_psum[:].rearrange("p (h e) -> p h e", e=NUM_EMB))
                    dqe = att.tile([128, H, NUM_EMB - 1], F32, tag="dqe")
                    nc.vector.tensor_tensor(out=dqe[:], in0=qe_sb[:, :, 1:],
                                            in1=qe_sb[:, :, :-1], op=ALU.subtract)

                    if stage == "masks":
                        if not os.environ.get("NODUMP"):
                            md = att.tile([128, S], F32, tag="md")
                            nc.vector.tensor_copy(md[:], masks[:, 0, :])
                            nc.sync.dma_start(out=out[0:128, 0:D], in_=md[:, 0:D])
                        nc.sync.dma_start(out=out[128:256, 0:D], in_=d2[:, 0:D])
                        continue

                    for h in range(H if (stage != "logits" or os.environ.get("FULLLOOPS")) else 1):
                        off = 0 if os.environ.get("OFF0") else (64 * h) % 128
                        # --- logits = q.k^T ---
                        qk = qk_ps.tile([128, S], F32, tag="qk")
                        for hf in range(2):
                            nc.tensor.matmul(qk[:, 512 * hf:512 * hf + 512],
                                             qT[off:off + 64, h // 2, 128 * qt:128 * qt + 128],
                                             kT[off:off + 64, h // 2, 512 * hf:512 * hf + 512],
                                             start=True, stop=True)
                        # --- + bias: 9 chained masked MACs ---
                        src = qk
                        if stage == "qkonly":
                            o3 = att.tile([128, DK], F32, tag="o")
                            nc.scalar.copy(o3[:], qk[:, 0:DK])
                            nc.sync.dma_start(out=out[128 * qt:128 * qt + 128, 64 * h:64 * h + 64],
                                              in_=o3[:])
                            continue
        
                        nstt = 0 if stage == "qkexp" else (NUM_EMB - 1)
                        for e in range(nstt):
                            acc = accp.tile([128, S], F32, tag="acc")
                            nc.vector.scalar_tensor_tensor(
                                out=acc[:], in0=masks[:, e, :], scalar=dqe[:, h, e:e + 1],
                                in1=src[:], op0=ALU.mult, op1=ALU.add)
                            src = acc
                        if stage == "sttonly":
                            o3 = att.tile([128, DK], F32, tag="o")
                            nc.vector.tensor_copy(o3[:], src[:, 0:DK])
                            nc.sync.dma_start(out=out[128 * qt:128 * qt + 128, 64 * h:64 * h + 64],
                                              in_=o3[:])
                            continue
                        # --- P = exp(logits/8), row-sum, transpose ---
                        p_sb = att.tile([128, S], BF16, tag="p")
                        den = att.tile([128, 1], F32, tag="den")
                        nc.scalar.activation(p_sb[:], src[:], ACT.Exp, scale=0.125,
                                             accum_out=den[:])
                        if stage in ("logits", "qkexp"):
                            pf = att.tile([128, S], F32, tag="pf")
                            nc.vector.tensor_copy(pf[:], p_sb[:])
                            nc.sync.dma_start(out=out[0:128, 0:D], in_=pf[:, 0:D])
                            continue
                        if os.environ.get("PSTAGE"):
                            p2 = att.tile([128, S], BF16, tag="p2")
                            nc.vector.tensor_copy(p2[:], p_sb[:])
                            p_sb = p2
                        pT = att.tile([128, NKT, 128], BF16, tag="pT")
                        if stage in ("notrans", "nopt", "av"):
                            nc.sync.dma_start(out=pT[:], in_=p_sb[:].rearrange("p (c j) -> p c j", j=128))
                        elif os.environ.get("XBARTRANS"):
                            # xbar transpose is only correct up to 512-wide inputs;
                            # first-op-in-kernel also glitches (see PE path below)
                            nc.sync.dma_start_transpose(pT[:, 0:NKT // 2, :], p_sb[:, 0:S // 2])
                            nc.sync.dma_start_transpose(pT[:, NKT // 2:NKT, :], p_sb[:, S // 2:S])
                        else:
                            ptp = pt_ps.tile([128, NKT, 128], BF16, tag="ptp")
                            for c in range(NKT):
                                nc.tensor.transpose(ptp[:, c, :], p_sb[:, 128 * c:128 * c + 128], ident[:])
                            nc.scalar.copy(pT[:], ptp[:])
                        # --- out_h = (P^T . V_h) / den + bv_h ---
                        if stage == "nopt":
                            # skip everything after exp except a pT dump
                            o2 = att.tile([128, DK], F32, tag="o")
                            nc.vector.tensor_copy(o2[:], pT[:, 0, 0:DK])
                            nc.sync.dma_start(out=out[128 * qt:128 * qt + 128, 64 * h:64 * h + 64],
                                              in_=o2[:])
                            continue
                        if os.environ.get("PTCOPY"):
                            pT2 = att.tile([128, NKT, 128], BF16, tag="pT2")
                            nc.vector.tensor_copy(pT2[:], pT[:])
                            pT = pT2
                        av = av_ps.tile([128, DK], F32, tag="av")
                        for c in range(NKT):
                            nc.tensor.matmul(av[:], pT[:, c, :], v_sb[:, c, 64 * h:64 * h + 64],
                                             start=(c == 0), stop=(c == NKT - 1))
                        if stage == "av":
                            o2 = att.tile([128, DK], F32, tag="o")
                            nc.scalar.copy(o2[:], av[:])
                            nc.sync.dma_start(out=out[128 * qt:128 * qt + 128, 64 * h:64 * h + 64],
                                              in_=o2[:])
                            continue
                        recip = att.tile([128, 1], F32, tag="recip")
                        nc.vector.reciprocal(recip[:], den[:])
                        o_sb = att.tile([128, DK], F32, tag="o")
                        nc.vector.scalar_tensor_tensor(
                            out=o_sb[:], in0=av[:], scalar=recip[:],
                            in1=bv_b[:, 64 * h:64 * h + 64], op0=ALU.mult, op1=ALU.add)
                        nc.sync.dma_start(out=out[128 * qt:128 * qt + 128, 64 * h:64 * h + 64],
                                          in_=o_sb[:])
    nc.compile()
    return nc


_NC_CACHE = {}


def _get_nc():
    if "nc" not in _NC_CACHE:
        _NC_CACHE["nc"] = build_nc()
    return _NC_CACHE["nc"]


def kernel(query, key, value, tile_positions, Wq, bq, Wk, bk, Wv, bv, emb_k):
    query = np.ascontiguousarray(np.asarray(query, dtype=np.float32))
    key = np.ascontiguousarray(np.asarray(key, dtype=np.float32))
    value = np.ascontiguousarray(np.asarray(value, dtype=np.float32))
    tile_positions = np.ascontiguousarray(np.asarray(tile_positions, dtype=np.float32))
    Wq = np.ascontiguousarray(np.asarray(Wq, dtype=np.float32))
    Wk = np.ascontiguousarray(np.asarray(Wk, dtype=np.float32))
    Wv = np.ascontiguousarray(np.asarray(Wv, dtype=np.float32))
    bq = np.ascontiguousarray(np.asarray(bq, dtype=np.float32))
    bk = np.ascontiguousarray(np.asarray(bk, dtype=np.float32))
    bv = np.ascontiguousarray(np.asarray(bv, dtype=np.float32))
    emb_k = np.ascontiguousarray(np.asarray(emb_k, dtype=np.float32))

    nc = _get_nc()
    in_maps = []
    for c in range(NCORES):
        b, qh = c // 2, c % 2
        in_maps.append({
            "xq": np.ascontiguousarray(query[b, qh * SQ:(qh + 1) * SQ]),
            "xk": key[b], "xv": value[b],
            "pos": tile_positions[b],
            "posq": np.ascontiguousarray(tile_positions[b, qh * SQ:(qh + 1) * SQ]),
            "wq": Wq, "wk": Wk, "wv": Wv,
            "bq": bq, "bk": bk, "bv": bv,
            "emb": emb_k,
        })
    res = run_bass_kernel_spmd(nc, in_maps, core_ids=list(range(NCORES)))
    out = np.empty((B, S, D), np.float32)
    for c in range(NCORES):
        b, qh = c // 2, c % 2
        out[b, qh * SQ:(qh + 1) * SQ] = res.results[c]["out"]
    return out



# revision 3
# speedup vs baseline: 7.6992x; 7.6992x over previous
"""Distance-aware multihead attention on 8 Trainium2 NeuronCores (v2).

Problem: B=4, S=1024, D=768, H=12, DK=64, NUM_EMB=10.
  q/k/v = linear projections of query/key/value
  idx[b,i,j] = clip(round(9 * |pos_i - pos_j| / MAXD), 0, 9)
  logits = (q.k^T + qe[b,h,i,idx[b,i,j]]) / 8   where qe = q @ emb_k^T
  out = softmax(logits) @ v

Key decompositions:
  - bias qe[...,idx] = sum_{e=1..9} (qe_e - qe_{e-1}) * (d2 >= T_e^2); the
    qe_0 term is constant along the softmax axis and cancels.
  - step masks (d2 >= T_e^2) are shared across all 12 heads of a q-tile.
  - bias applied on the TENSOR engine: 9 diag(dqe_e) @ mask_e matmuls
    accumulated into the same PSUM group as the QK matmul (diag matmul ==
    per-partition row scaling).  This removes the per-head DVE STT chain.

v2 changes vs v1: natural (contiguous) DMA loads spread over the SP/ACT/Pool
queues with on-chip PE transposition (the v1 xbar-transposed loads ran at
~6 GB/s and serialized 2.5 ms); fp16 operands everywhere on the PE except
bf16 P/V; software-pipelined head loop (PE group h+1 issued before the
transpose/AV tail of head h).

Sharding: core c handles batch c//2, query-half c%2 (512 queries, all heads).
"""
import numpy as np

import concourse.bass as bass
import concourse.tile as tile
from concourse import bacc, mybir
from concourse.bass_utils import run_bass_kernel_spmd

F32 = mybir.dt.float32
BF16 = mybir.dt.bfloat16
FP16 = mybir.dt.float16
ACT = mybir.ActivationFunctionType
ALU = mybir.AluOpType

B, S, D = 4, 1024, 768
H, DK = 12, 64
NUM_EMB = 10
MAX_DIST = 100000.0 * 2 ** 0.5
SQ = S // 2          # queries per core
NQT = SQ // 128      # q-tiles per core (4)
NKT = S // 128       # k token chunks (8)
NDT = D // 128       # dim tiles (6)
NCORES = 8

# squared thresholds: idx >= e  <=>  d2 >= ((e-0.5)*MAX_DIST/9)^2
THRESH2 = [float(((e - 0.5) * MAX_DIST / 9.0) ** 2) for e in range(1, NUM_EMB)]


def build_nc():
    nc = bacc.Bacc("TRN2", target_bir_lowering=False, debug=False)

    xq = nc.dram_tensor("xq", [SQ, D], F32, kind="ExternalInput").ap()
    xk = nc.dram_tensor("xk", [S, D], F32, kind="ExternalInput").ap()
    xv = nc.dram_tensor("xv", [S, D], F32, kind="ExternalInput").ap()
    pos = nc.dram_tensor("pos", [S, 2], F32, kind="ExternalInput").ap()
    posq = nc.dram_tensor("posq", [SQ, 2], F32, kind="ExternalInput").ap()
    wq = nc.dram_tensor("wq", [D, D], F32, kind="ExternalInput").ap()
    wk = nc.dram_tensor("wk", [D, D], F32, kind="ExternalInput").ap()
    wv = nc.dram_tensor("wv", [D, D], F32, kind="ExternalInput").ap()
    bq = nc.dram_tensor("bq", [D], F32, kind="ExternalInput").ap()
    bk = nc.dram_tensor("bk", [D], F32, kind="ExternalInput").ap()
    bv = nc.dram_tensor("bv", [D], F32, kind="ExternalInput").ap()
    emb = nc.dram_tensor("emb", [NUM_EMB, DK], F32, kind="ExternalInput").ap()
    out = nc.dram_tensor("out", [SQ, D], F32, kind="ExternalOutput").ap()

    with tile.TileContext(nc) as tc:
        with tc.tile_pool(name="persist", bufs=1) as persist:
            from concourse.masks import make_identity
            ident16 = persist.tile([128, 128], FP16)
            ident32 = persist.tile([128, 128], F32)
            identb = persist.tile([128, 128], BF16)
            make_identity(nc, ident16[:])
            make_identity(nc, ident32[:])
            make_identity(nc, identb[:])

            # small loads on the Pool SWDGE queue
            bq_col = persist.tile([128, NDT], F32)
            bk_col = persist.tile([128, NDT], F32)
            nc.gpsimd.dma_start(out=bq_col[:], in_=bass.AP(tensor=bq.tensor, offset=0, ap=[[1, 128], [128, NDT]]))
            nc.gpsimd.dma_start(out=bk_col[:], in_=bass.AP(tensor=bk.tensor, offset=0, ap=[[1, 128], [128, NDT]]))
            bv_b = persist.tile([128, D], F32)
            nc.gpsimd.dma_start(out=bv_b[:], in_=bass.AP(tensor=bv.tensor, offset=0, ap=[[0, 128], [1, D]]))
            # position x/y as single-partition rows [1, S]
            posx_row = persist.tile([1, S], F32)
            posy_row = persist.tile([1, S], F32)
            nc.gpsimd.dma_start(out=posx_row[:], in_=bass.AP(tensor=pos.tensor, offset=0, ap=[[2, 1], [2, S]]))
            nc.gpsimd.dma_start(out=posy_row[:], in_=bass.AP(tensor=pos.tensor, offset=1, ap=[[2, 1], [2, S]]))
            # query positions as per-partition scalars [128, NQT]
            xq_col = persist.tile([128, NQT], F32)
            yq_col = persist.tile([128, NQT], F32)
            nc.gpsimd.dma_start(out=xq_col[:], in_=bass.AP(tensor=posq.tensor, offset=0, ap=[[2, 128], [256, NQT]]))
            nc.gpsimd.dma_start(out=yq_col[:], in_=bass.AP(tensor=posq.tensor, offset=1, ap=[[2, 128], [256, NQT]]))
            # emb^T block-diagonal [128, 2*NUM_EMB] fp16 (2 heads per matmul)
            embT_f = persist.tile([64, NUM_EMB], F32)
            nc.sync.dma_start_transpose(embT_f[:, :], emb[:, :])
            embT_blk = persist.tile([128, 2 * NUM_EMB], FP16)
            nc.vector.memset(embT_blk[:], 0.0)
            nc.vector.tensor_copy(embT_blk[0:64, 0:NUM_EMB], embT_f[:, :])
            nc.vector.tensor_copy(embT_blk[64:128, NUM_EMB:2 * NUM_EMB], embT_f[:, :])

            ones1 = persist.tile([1, 128], F32)
            nc.vector.memset(ones1[:], 1.0)

            # persistent attention operands
            kT = persist.tile([128, NDT, S], FP16)      # K^T[dim, token] + bk
            qT = persist.tile([128, NDT, SQ], FP16)     # Q^T[dim, token] + bq
            v_sb = persist.tile([128, NKT, D], BF16)    # V[token, dim] (no bias)
            xk_b = persist.tile([128, S], F32)          # pos-x broadcast rows
            yk_b = persist.tile([128, S], F32)
            dqe = persist.tile([128, NQT, H, NUM_EMB - 1], F32)

            # ---- load + transpose + project ----
            with tc.tile_pool(name="nat", bufs=1) as natp, \
                 tc.tile_pool(name="tsp", bufs=1) as tsp, \
                 tc.tile_pool(name="tp_ps", bufs=2, space="PSUM") as tp_ps, \
                 tc.tile_pool(name="pj_ps", bufs=2, space="PSUM") as pj_ps, \
                 tc.tile_pool(name="bc_ps", bufs=1, space="PSUM") as bc_ps, \
                 tc.tile_pool(name="qe_ps", bufs=1, space="PSUM") as qe_ps:

                # broadcast pos rows across partitions via 1-partition matmul
                for dst, row in ((xk_b, posx_row), (yk_b, posy_row)):
                    for hf in range(2):
                        sl = slice(512 * hf, 512 * hf + 512)
                        bc = bc_ps.tile([128, 512], F32, tag="bc")
                        nc.tensor.matmul(bc[:], ones1[:], row[:, sl],
                                         start=True, stop=True)
                        nc.scalar.copy(dst[:, sl], bc[:])

                def load_transpose(src, ncols, dst, dma_eng, nchunks):
                    """src [nchunks*128, ncols*128] DRAM -> dst [128, ncols, nchunks*128]
                    fp16 = src^T, via natural chunk loads + PE transposes."""
                    nat = natp.tile([128, NKT, D], F32, tag="nat", name=f"nat_{src.tensor.name}")
                    for c in range(nchunks):
                        dma_eng.dma_start(out=nat[:, c, 0:ncols * 128],
                                          in_=src[128 * c:128 * (c + 1), :])
                    for d in range(ncols):
                        for u in range((nchunks + 3) // 4):
                            nb = min(4, nchunks - 4 * u)
                            ps = tp_ps.tile([128, 4, 128], F32, tag="tp")
                            for c in range(nb):
                                nc.tensor.transpose(ps[:, c, :],
                                                    nat[:, 4 * u + c, 128 * d:128 * d + 128],
                                                    ident32[:])
                            nc.scalar.copy(
                                dst[:, d, 512 * u:512 * u + 128 * nb],
                                ps[:, 0:nb, :].rearrange("p c f -> p (c f)"))

                # V first
                xvT = tsp.tile([128, NDT, S], FP16, tag="xT")
                wvT = tsp.tile([128, NDT, D], FP16, tag="wT")
                load_transpose(xv, NDT, xvT, nc.sync, NKT)
                load_transpose(wv, NDT, wvT, nc.scalar, NDT)
                for m in range(NKT):
                    for hf in range(2):
                        ps = pj_ps.tile([128, 512], F32, tag="pj")
                        for t in range(NDT):
                            nc.tensor.matmul(ps[:, 0:384], xvT[:, t, 128 * m:128 * m + 128],
                                             wvT[:, t, 384 * hf:384 * hf + 384],
                                             start=(t == 0), stop=(t == NDT - 1))
                        nc.scalar.copy(v_sb[:, m, 384 * hf:384 * hf + 384], ps[:, 0:384])

                # K
                xkT = tsp.tile([128, NDT, S], FP16, tag="xT")
                wkT = tsp.tile([128, NDT, D], FP16, tag="wT")
                load_transpose(xk, NDT, xkT, nc.sync, NKT)
                load_transpose(wk, NDT, wkT, nc.scalar, NDT)
                for m in range(NDT):
                    for hf in range(2):
                        ps = pj_ps.tile([128, 512], F32, tag="pj")
                        for t in range(NDT):
                            nc.tensor.matmul(ps[:], wkT[:, t, 128 * m:128 * m + 128],
                                             xkT[:, t, 512 * hf:512 * hf + 512],
                                             start=(t == 0), stop=(t == NDT - 1))
                        nc.scalar.activation(kT[:, m, 512 * hf:512 * hf + 512], ps[:],
                                             ACT.Identity, bias=bk_col[:, m:m + 1])

                # Q
                xqT = tsp.tile([128, NDT, S], FP16, tag="xT")
                wqT = tsp.tile([128, NDT, D], FP16, tag="wT")
                load_transpose(xq, NDT, xqT, nc.sync, NQT)
                load_transpose(wq, NDT, wqT, nc.scalar, NDT)
                for m in range(NDT):
                    ps = pj_ps.tile([128, 512], F32, tag="pj")
                    for t in range(NDT):
                        nc.tensor.matmul(ps[:], wqT[:, t, 128 * m:128 * m + 128],
                                         xqT[:, t, 0:SQ],
                                         start=(t == 0), stop=(t == NDT - 1))
                    nc.scalar.activation(qT[:, m, :], ps[:], ACT.Identity,
                                         bias=bq_col[:, m:m + 1])

                # qe for all q-tiles: block-diag emb matmul, 2 heads per 128-dim block
                qe_psum = qe_ps.tile([128, NQT, H * NUM_EMB], F32)
                for qt in range(NQT):
                    for m in range(NDT):
                        nc.tensor.matmul(qe_psum[:, qt, 20 * m:20 * m + 20],
                                         qT[:, m, 128 * qt:128 * qt + 128],
                                         embT_blk[:],
                                         start=True, stop=True)
                qe_sb = persist.tile([128, NQT, H, NUM_EMB], F32)
                nc.scalar.copy(qe_sb[:], qe_psum[:].rearrange("p q (h e) -> p q h e", e=NUM_EMB))
                nc.vector.tensor_tensor(out=dqe[:], in0=qe_sb[:, :, :, 1:],
                                        in1=qe_sb[:, :, :, :-1], op=ALU.subtract)

            # ---- attention ----
            with tc.tile_pool(name="att", bufs=2) as att, \
                 tc.tile_pool(name="prep", bufs=2) as prep, \
                 tc.tile_pool(name="qk_ps", bufs=2, space="PSUM") as qk_ps, \
                 tc.tile_pool(name="pt_ps", bufs=2, space="PSUM") as pt_ps, \
                 tc.tile_pool(name="av_ps", bufs=2, space="PSUM") as av_ps:

                def emit_prep(qt):
                    # d2 = |pos_k - pos_q|^2 for this q-tile, then 9 step masks
                    dx = prep.tile([128, S], F32, tag="dx")
                    dy = prep.tile([128, S], F32, tag="dy")
                    nc.vector.tensor_scalar(out=dx[:], in0=xk_b[:], scalar1=xq_col[:, qt:qt + 1],
                                            scalar2=None, op0=ALU.subtract)
                    nc.vector.tensor_scalar(out=dy[:], in0=yk_b[:], scalar1=yq_col[:, qt:qt + 1],
                                            scalar2=None, op0=ALU.subtract)
                    dx2 = prep.tile([128, S], F32, tag="dx2")
                    dy2 = prep.tile([128, S], F32, tag="dy2")
                    nc.scalar.square(dx2[:], dx[:])
                    nc.scalar.square(dy2[:], dy[:])
                    d2 = prep.tile([128, S], F32, tag="d2")
                    nc.vector.tensor_add(d2[:], dx2[:], dy2[:])
                    masks = prep.tile([128, NUM_EMB - 1, S], FP16, tag="masks")
                    for e in range(NUM_EMB - 1):
                        nc.vector.tensor_scalar(out=masks[:, e, :], in0=d2[:],
                                                scalar1=THRESH2[e], scalar2=None,
                                                op0=ALU.is_ge)
                    return masks

                masks_by_qt = {0: emit_prep(0)}

                def emit_front(qt, h, masks):
                    # DVE: 9 per-head diag(dqe) builds
                    diag = att.tile([128, NUM_EMB - 1, 128], FP16, tag="diag")
                    for e in range(NUM_EMB - 1):
                        nc.vector.tensor_scalar(out=diag[:, e, :], in0=ident16[:],
                                                scalar1=dqe[:, qt, h, e:e + 1],
                                                scalar2=None, op0=ALU.mult)
                    # PE: qk + bias into one PSUM group per 512-half
                    off = (64 * h) % 128
                    qk = qk_ps.tile([128, S], F32, tag="qk")
                    for hf in range(2):
                        sl = slice(512 * hf, 512 * hf + 512)
                        nc.tensor.matmul(qk[:, sl],
                                         qT[off:off + 64, h // 2, 128 * qt:128 * qt + 128],
                                         kT[off:off + 64, h // 2, sl],
                                         start=True, stop=False)
                        for e in range(NUM_EMB - 1):
                            nc.tensor.matmul(qk[:, sl], diag[:, e, :], masks[:, e, sl],
                                             start=False, stop=(e == NUM_EMB - 2))
                    # ACT: P = exp(logits/8) bf16 + row-sum
                    p_sb = att.tile([128, S], BF16, tag="p")
                    den = att.tile([128, 1], F32, tag="den")
                    nc.scalar.activation(p_sb[:], qk[:], ACT.Exp, scale=0.125,
                                         accum_out=den[:])
                    return p_sb, den

                def emit_tail(qt, h, p_sb, den, o_parts):
                    # PE: transpose P to [k, q] chunks; ACT: evacuate
                    ptp = pt_ps.tile([128, NKT, 128], BF16, tag="ptp")
                    for c in range(NKT):
                        nc.tensor.transpose(ptp[:, c, :], p_sb[:, 128 * c:128 * c + 128],
                                            identb[:])
                    pT = att.tile([128, NKT, 128], BF16, tag="pT")
                    nc.scalar.copy(pT[:], ptp[:])
                    # PE: AV accumulate over k chunks
                    av = av_ps.tile([128, DK], F32, tag="av")
                    for c in range(NKT):
                        nc.tensor.matmul(av[:], pT[:, c, :], v_sb[:, c, 64 * h:64 * h + 64],
                                         start=(c == 0), stop=(c == NKT - 1))
                    # DVE: out_h = av/den + bv_h
                    recip = att.tile([128, 1], F32, tag="recip")
                    nc.vector.reciprocal(recip[:], den[:])
                    nc.vector.scalar_tensor_tensor(
                        out=o_parts[:, h, :], in0=av[:], scalar=recip[:],
                        in1=bv_b[:, 64 * h:64 * h + 64], op0=ALU.mult, op1=ALU.add)

                pending = None  # (qt, h, p_sb, den, o_parts)
                o_parts = None
                for qt in range(NQT):
                    o_parts = att.tile([128, H, DK], F32, tag="o", name=f"o_{qt}")
                    masks = masks_by_qt.pop(qt)
                    for h in range(H):
                        front = emit_front(qt, h, masks)
                        if h == 0 and qt + 1 < NQT:
                            masks_by_qt[qt + 1] = emit_prep(qt + 1)
                        if pending is not None:
                            pq, ph, pp, pd, po = pending
                            emit_tail(pq, ph, pp, pd, po)
                            if ph == H - 1:
                                nc.sync.dma_start(
                                    out=out[128 * pq:128 * pq + 128, :],
                                    in_=po[:].rearrange("p h d -> p (h d)"))
                        pending = (qt, h) + front + (o_parts,)
                pq, ph, pp, pd, po = pending
                emit_tail(pq, ph, pp, pd, po)
                nc.sync.dma_start(out=out[128 * pq:128 * pq + 128, :],
                                  in_=po[:].rearrange("p h d -> p (h d)"))
    nc.compile()
    return nc


_NC_CACHE = {}


def _get_nc():
    if "nc" not in _NC_CACHE:
        _NC_CACHE["nc"] = build_nc()
    return _NC_CACHE["nc"]


def _make_in_maps(inputs):
    q = np.ascontiguousarray(np.asarray(inputs["query"], dtype=np.float32))
    k = np.ascontiguousarray(np.asarray(inputs["key"], dtype=np.float32))
    v = np.ascontiguousarray(np.asarray(inputs["value"], dtype=np.float32))
    tp = np.ascontiguousarray(np.asarray(inputs["tile_positions"], dtype=np.float32))
    ws = {n: np.ascontiguousarray(np.asarray(inputs[n], dtype=np.float32))
          for n in ("Wq", "Wk", "Wv", "bq", "bk", "bv", "emb_k")}
    in_maps = []
    for c in range(NCORES):
        b, qh = c // 2, c % 2
        in_maps.append({
            "xq": np.ascontiguousarray(q[b, qh * SQ:(qh + 1) * SQ]),
            "xk": k[b], "xv": v[b],
            "pos": tp[b],
            "posq": np.ascontiguousarray(tp[b, qh * SQ:(qh + 1) * SQ]),
            "wq": ws["Wq"], "wk": ws["Wk"], "wv": ws["Wv"],
            "bq": ws["bq"], "bk": ws["bk"], "bv": ws["bv"],
            "emb": ws["emb_k"],
        })
    return in_maps


def kernel(query, key, value, tile_positions, Wq, bq, Wk, bk, Wv, bv, emb_k):
    inputs = {"query": query, "key": key, "value": value,
              "tile_positions": tile_positions,
              "Wq": Wq, "bq": bq, "Wk": Wk, "bk": bk, "Wv": Wv, "bv": bv,
              "emb_k": emb_k}
    nc = _get_nc()
    in_maps = _make_in_maps(inputs)
    res = run_bass_kernel_spmd(nc, in_maps, core_ids=list(range(NCORES)))
    out = np.empty((B, S, D), np.float32)
    for c in range(NCORES):
        b, qh = c // 2, c % 2
        out[b, qh * SQ:(qh + 1) * SQ] = res.results[c]["out"]
    return out


# revision 4
# speedup vs baseline: 9.1970x; 1.1945x over previous
"""Distance-aware multihead attention on 8 Trainium2 NeuronCores (v3).

Problem: B=4, S=1024, D=768, H=12, DK=64, NUM_EMB=10.
  q/k/v = linear projections of query/key/value
  idx[b,i,j] = clip(round(9 * |pos_i - pos_j| / MAXD), 0, 9)
  logits = (q.k^T + qe[b,h,i,idx[b,i,j]]) / 8   where qe = q @ emb_k^T
  out = softmax(logits) @ v

Key decompositions:
  - bias qe[...,idx] = sum_{e=1..9} (qe_e - qe_{e-1}) * (d2 >= T_e^2); the
    qe_0 term is constant along the softmax axis and cancels.
  - step masks (d2 >= T_e^2) are shared across all 12 heads of a q-tile.
  - bias applied on the TENSOR engine: 9 diag(dqe_e) @ mask_e matmuls
    accumulated into the same PSUM group as the QK matmul (a diag matmul is
    a per-partition row scaling).  No per-element vector work for the bias.

v3: X and W arrive from the host already transposed ([in_dim, token] /
[in_dim, out_dim]) and cast to fp16, so DMA loads land in the exact SBUF
layout the PE needs - no on-chip transposition of inputs at all.  All PE
operands fp16 except P/V (bf16 for exp range).  Head loop is software-
pipelined (PE group for head h+1 issues before the transpose/AV tail of
head h).

Sharding: core c handles batch c//2, query-half c%2 (512 queries, all heads).
"""
import numpy as np

import concourse.bass as bass
import concourse.tile as tile
from concourse import bacc, mybir
from concourse.bass_utils import run_bass_kernel_spmd

F32 = mybir.dt.float32
BF16 = mybir.dt.bfloat16
FP16 = mybir.dt.float16
ACT = mybir.ActivationFunctionType
ALU = mybir.AluOpType

B, S, D = 4, 1024, 768
H, DK = 12, 64
NUM_EMB = 10
MAX_DIST = 100000.0 * 2 ** 0.5
SQ = S // 2          # queries per core
NQT = SQ // 128      # q-tiles per core (4)
NKT = S // 128       # k token chunks (8)
NDT = D // 128       # dim tiles (6)
NCORES = 8

# squared thresholds: idx >= e  <=>  d2 >= ((e-0.5)*MAX_DIST/9)^2
THRESH2 = [float(((e - 0.5) * MAX_DIST / 9.0) ** 2) for e in range(1, NUM_EMB)]


def build_nc():
    nc = bacc.Bacc("TRN2", target_bir_lowering=False, debug=False)

    # host-pretransposed fp16 operands: [contraction_dim, free_dim]
    xqT_d = nc.dram_tensor("xqT", [D, SQ], FP16, kind="ExternalInput").ap()
    xkT_d = nc.dram_tensor("xkT", [D, S], FP16, kind="ExternalInput").ap()
    xvT_d = nc.dram_tensor("xvT", [D, S], FP16, kind="ExternalInput").ap()
    wqT_d = nc.dram_tensor("wqT", [D, D], FP16, kind="ExternalInput").ap()
    wkT_d = nc.dram_tensor("wkT", [D, D], FP16, kind="ExternalInput").ap()
    wvT_d = nc.dram_tensor("wvT", [D, D], FP16, kind="ExternalInput").ap()
    embT_d = nc.dram_tensor("embT", [DK, NUM_EMB], FP16, kind="ExternalInput").ap()
    pos = nc.dram_tensor("pos", [S, 2], F32, kind="ExternalInput").ap()
    posq = nc.dram_tensor("posq", [SQ, 2], F32, kind="ExternalInput").ap()
    bq = nc.dram_tensor("bq", [D], F32, kind="ExternalInput").ap()
    bk = nc.dram_tensor("bk", [D], F32, kind="ExternalInput").ap()
    bv = nc.dram_tensor("bv", [D], F32, kind="ExternalInput").ap()
    out = nc.dram_tensor("out", [SQ, D], F32, kind="ExternalOutput").ap()

    with tile.TileContext(nc) as tc:
        with tc.tile_pool(name="persist", bufs=1) as persist:
            from concourse.masks import make_identity
            ident16 = persist.tile([128, 128], FP16)
            identb = persist.tile([128, 128], BF16)
            make_identity(nc, ident16[:])
            make_identity(nc, identb[:])

            # small loads on the Pool SWDGE queue
            bq_col = persist.tile([128, NDT], F32)
            bk_col = persist.tile([128, NDT], F32)
            nc.gpsimd.dma_start(out=bq_col[:], in_=bass.AP(tensor=bq.tensor, offset=0, ap=[[1, 128], [128, NDT]]))
            nc.gpsimd.dma_start(out=bk_col[:], in_=bass.AP(tensor=bk.tensor, offset=0, ap=[[1, 128], [128, NDT]]))
            bv_b = persist.tile([128, D], F32)
            nc.gpsimd.dma_start(out=bv_b[:], in_=bass.AP(tensor=bv.tensor, offset=0, ap=[[0, 128], [1, D]]))
            # position x/y as single-partition rows [1, S]
            posx_row = persist.tile([1, S], F32)
            posy_row = persist.tile([1, S], F32)
            nc.gpsimd.dma_start(out=posx_row[:], in_=bass.AP(tensor=pos.tensor, offset=0, ap=[[2, 1], [2, S]]))
            nc.gpsimd.dma_start(out=posy_row[:], in_=bass.AP(tensor=pos.tensor, offset=1, ap=[[2, 1], [2, S]]))
            # query positions as per-partition scalars [128, NQT]
            xq_col = persist.tile([128, NQT], F32)
            yq_col = persist.tile([128, NQT], F32)
            nc.gpsimd.dma_start(out=xq_col[:], in_=bass.AP(tensor=posq.tensor, offset=0, ap=[[2, 128], [256, NQT]]))
            nc.gpsimd.dma_start(out=yq_col[:], in_=bass.AP(tensor=posq.tensor, offset=1, ap=[[2, 128], [256, NQT]]))
            # emb^T block-diagonal [128, 2*NUM_EMB] fp16 (2 heads per matmul)
            embT_blk = persist.tile([128, 2 * NUM_EMB], FP16)
            nc.vector.memset(embT_blk[:], 0.0)
            nc.gpsimd.dma_start(out=embT_blk[0:64, 0:NUM_EMB], in_=embT_d[:, :])
            nc.gpsimd.dma_start(out=embT_blk[64:128, NUM_EMB:2 * NUM_EMB], in_=embT_d[:, :])

            ones1 = persist.tile([1, 128], F32)
            nc.vector.memset(ones1[:], 1.0)

            # persistent attention operands
            kT = persist.tile([128, NDT, S], FP16)      # K^T[dim, token] + bk
            qT = persist.tile([128, NDT, SQ], FP16)     # Q^T[dim, token] + bq
            v_sb = persist.tile([128, NKT, D], BF16)    # V[token, dim] (no bias)
            xk_b = persist.tile([128, S], F32)          # pos-x broadcast rows
            yk_b = persist.tile([128, S], F32)
            dqe = persist.tile([128, NQT, H, NUM_EMB - 1], F32)

            # ---- load (already transposed) + project ----
            with tc.tile_pool(name="tsp", bufs=2) as tsp, \
                 tc.tile_pool(name="pj_ps", bufs=2, space="PSUM") as pj_ps, \
                 tc.tile_pool(name="bc_ps", bufs=1, space="PSUM") as bc_ps, \
                 tc.tile_pool(name="qe_ps", bufs=1, space="PSUM") as qe_ps:

                # broadcast pos rows across partitions via 1-partition matmul
                for dst, row in ((xk_b, posx_row), (yk_b, posy_row)):
                    for hf in range(2):
                        sl = slice(512 * hf, 512 * hf + 512)
                        bc = bc_ps.tile([128, 512], F32, tag="bc")
                        nc.tensor.matmul(bc[:], ones1[:], row[:, sl],
                                         start=True, stop=True)
                        nc.scalar.copy(dst[:, sl], bc[:])

                def load_T(src, dst, ncols, dma_eng):
                    for d in range(NDT):
                        dma_eng.dma_start(out=dst[:, d, 0:ncols],
                                          in_=src[128 * d:128 * (d + 1), :])

                # V
                xvT = tsp.tile([128, NDT, S], FP16, tag="xT")
                wvT = tsp.tile([128, NDT, D], FP16, tag="wT")
                load_T(xvT_d, xvT, S, nc.sync)
                load_T(wvT_d, wvT, D, nc.scalar)
                for m in range(NKT):
                    for hf in range(2):
                        ps = pj_ps.tile([128, 512], F32, tag="pj")
                        for t in range(NDT):
                            nc.tensor.matmul(ps[:, 0:384], xvT[:, t, 128 * m:128 * m + 128],
                                             wvT[:, t, 384 * hf:384 * hf + 384],
                                             start=(t == 0), stop=(t == NDT - 1))
                        nc.scalar.copy(v_sb[:, m, 384 * hf:384 * hf + 384], ps[:, 0:384])

                # K
                xkT = tsp.tile([128, NDT, S], FP16, tag="xT")
                wkT = tsp.tile([128, NDT, D], FP16, tag="wT")
                load_T(xkT_d, xkT, S, nc.sync)
                load_T(wkT_d, wkT, D, nc.scalar)
                for m in range(NDT):
                    for hf in range(2):
                        ps = pj_ps.tile([128, 512], F32, tag="pj")
                        for t in range(NDT):
                            nc.tensor.matmul(ps[:], wkT[:, t, 128 * m:128 * m + 128],
                                             xkT[:, t, 512 * hf:512 * hf + 512],
                                             start=(t == 0), stop=(t == NDT - 1))
                        nc.scalar.activation(kT[:, m, 512 * hf:512 * hf + 512], ps[:],
                                             ACT.Identity, bias=bk_col[:, m:m + 1])

                # Q
                xqT = tsp.tile([128, NDT, S], FP16, tag="xT")
                wqT = tsp.tile([128, NDT, D], FP16, tag="wT")
                load_T(xqT_d, xqT, SQ, nc.sync)
                load_T(wqT_d, wqT, D, nc.scalar)
                for m in range(NDT):
                    ps = pj_ps.tile([128, 512], F32, tag="pj")
                    for t in range(NDT):
                        nc.tensor.matmul(ps[:], wqT[:, t, 128 * m:128 * m + 128],
                                         xqT[:, t, 0:SQ],
                                         start=(t == 0), stop=(t == NDT - 1))
                    nc.scalar.activation(qT[:, m, :], ps[:], ACT.Identity,
                                         bias=bq_col[:, m:m + 1])

                # qe for all q-tiles: block-diag emb matmul, 2 heads per 128-dim block
                qe_psum = qe_ps.tile([128, NQT, H * NUM_EMB], F32)
                for qt in range(NQT):
                    for m in range(NDT):
                        nc.tensor.matmul(qe_psum[:, qt, 20 * m:20 * m + 20],
                                         qT[:, m, 128 * qt:128 * qt + 128],
                                         embT_blk[:],
                                         start=True, stop=True)
                qe_sb = persist.tile([128, NQT, H, NUM_EMB], F32)
                nc.scalar.copy(qe_sb[:], qe_psum[:].rearrange("p q (h e) -> p q h e", e=NUM_EMB))
                nc.vector.tensor_tensor(out=dqe[:], in0=qe_sb[:, :, :, 1:],
                                        in1=qe_sb[:, :, :, :-1], op=ALU.subtract)

            # ---- attention ----
            with tc.tile_pool(name="att", bufs=2) as att, \
                 tc.tile_pool(name="prep", bufs=2) as prep, \
                 tc.tile_pool(name="qk_ps", bufs=2, space="PSUM") as qk_ps, \
                 tc.tile_pool(name="pt_ps", bufs=2, space="PSUM") as pt_ps, \
                 tc.tile_pool(name="av_ps", bufs=2, space="PSUM") as av_ps:

                def emit_prep(qt):
                    # d2 = |pos_k - pos_q|^2 for this q-tile, then 9 step masks
                    dx = prep.tile([128, S], F32, tag="dx")
                    dy = prep.tile([128, S], F32, tag="dy")
                    nc.vector.tensor_scalar(out=dx[:], in0=xk_b[:], scalar1=xq_col[:, qt:qt + 1],
                                            scalar2=None, op0=ALU.subtract)
                    nc.vector.tensor_scalar(out=dy[:], in0=yk_b[:], scalar1=yq_col[:, qt:qt + 1],
                                            scalar2=None, op0=ALU.subtract)
                    dx2 = prep.tile([128, S], F32, tag="dx2")
                    dy2 = prep.tile([128, S], F32, tag="dy2")
                    nc.scalar.square(dx2[:], dx[:])
                    nc.scalar.square(dy2[:], dy[:])
                    d2 = prep.tile([128, S], F32, tag="d2")
                    nc.vector.tensor_add(d2[:], dx2[:], dy2[:])
                    masks = prep.tile([128, NUM_EMB - 1, S], FP16, tag="masks")
                    for e in range(NUM_EMB - 1):
                        nc.vector.tensor_scalar(out=masks[:, e, :], in0=d2[:],
                                                scalar1=THRESH2[e], scalar2=None,
                                                op0=ALU.is_ge)
                    return masks

                masks_by_qt = {0: emit_prep(0)}

                def emit_front(qt, h, masks):
                    # DVE: 9 per-head diag(dqe) builds
                    diag = att.tile([128, NUM_EMB - 1, 128], FP16, tag="diag")
                    for e in range(NUM_EMB - 1):
                        nc.vector.tensor_scalar(out=diag[:, e, :], in0=ident16[:],
                                                scalar1=dqe[:, qt, h, e:e + 1],
                                                scalar2=None, op0=ALU.mult)
                    # PE: qk + bias into one PSUM group per 512-half
                    off = (64 * h) % 128
                    qk = qk_ps.tile([128, S], F32, tag="qk")
                    for hf in range(2):
                        sl = slice(512 * hf, 512 * hf + 512)
                        nc.tensor.matmul(qk[:, sl],
                                         qT[off:off + 64, h // 2, 128 * qt:128 * qt + 128],
                                         kT[off:off + 64, h // 2, sl],
                                         start=True, stop=False)
                        for e in range(NUM_EMB - 1):
                            nc.tensor.matmul(qk[:, sl], diag[:, e, :], masks[:, e, sl],
                                             start=False, stop=(e == NUM_EMB - 2))
                    # ACT: P = exp(logits/8) bf16 + row-sum
                    p_sb = att.tile([128, S], BF16, tag="p")
                    den = att.tile([128, 1], F32, tag="den")
                    nc.scalar.activation(p_sb[:], qk[:], ACT.Exp, scale=0.125,
                                         accum_out=den[:])
                    return p_sb, den

                def emit_tail(qt, h, p_sb, den, o_parts):
                    # PE: transpose P to [k, q] chunks; ACT: evacuate
                    ptp = pt_ps.tile([128, NKT, 128], BF16, tag="ptp")
                    for c in range(NKT):
                        nc.tensor.transpose(ptp[:, c, :], p_sb[:, 128 * c:128 * c + 128],
                                            identb[:])
                    pT = att.tile([128, NKT, 128], BF16, tag="pT")
                    nc.scalar.copy(pT[:], ptp[:])
                    # PE: AV accumulate over k chunks
                    av = av_ps.tile([128, DK], F32, tag="av")
                    for c in range(NKT):
                        nc.tensor.matmul(av[:], pT[:, c, :], v_sb[:, c, 64 * h:64 * h + 64],
                                         start=(c == 0), stop=(c == NKT - 1))
                    # DVE: out_h = av/den + bv_h
                    recip = att.tile([128, 1], F32, tag="recip")
                    nc.vector.reciprocal(recip[:], den[:])
                    nc.vector.scalar_tensor_tensor(
                        out=o_parts[:, h, :], in0=av[:], scalar=recip[:],
                        in1=bv_b[:, 64 * h:64 * h + 64], op0=ALU.mult, op1=ALU.add)

                pending = None  # (qt, h, p_sb, den, o_parts)
                o_parts = None
                for qt in range(NQT):
                    o_parts = att.tile([128, H, DK], F32, tag="o", name=f"o_{qt}")
                    masks = masks_by_qt.pop(qt)
                    for h in range(H):
                        front = emit_front(qt, h, masks)
                        if h == 0 and qt + 1 < NQT:
                            masks_by_qt[qt + 1] = emit_prep(qt + 1)
                        if pending is not None:
                            pq, ph, pp, pd, po = pending
                            emit_tail(pq, ph, pp, pd, po)
                            if ph == H - 1:
                                nc.sync.dma_start(
                                    out=out[128 * pq:128 * pq + 128, :],
                                    in_=po[:].rearrange("p h d -> p (h d)"))
                        pending = (qt, h) + front + (o_parts,)
                pq, ph, pp, pd, po = pending
                emit_tail(pq, ph, pp, pd, po)
                nc.sync.dma_start(out=out[128 * pq:128 * pq + 128, :],
                                  in_=po[:].rearrange("p h d -> p (h d)"))
    nc.compile()
    return nc


_NC_CACHE = {}


def _get_nc():
    if "nc" not in _NC_CACHE:
        _NC_CACHE["nc"] = build_nc()
    return _NC_CACHE["nc"]


def _make_in_maps(inputs):
    q = np.asarray(inputs["query"], dtype=np.float32)
    k = np.asarray(inputs["key"], dtype=np.float32)
    v = np.asarray(inputs["value"], dtype=np.float32)
    tp = np.ascontiguousarray(np.asarray(inputs["tile_positions"], dtype=np.float32))
    f16 = lambda a: np.ascontiguousarray(a.astype(np.float16))
    wqT = f16(np.asarray(inputs["Wq"], dtype=np.float32).T)
    wkT = f16(np.asarray(inputs["Wk"], dtype=np.float32).T)
    wvT = f16(np.asarray(inputs["Wv"], dtype=np.float32).T)
    embT = f16(np.asarray(inputs["emb_k"], dtype=np.float32).T)
    bqa = np.ascontiguousarray(np.asarray(inputs["bq"], dtype=np.float32))
    bka = np.ascontiguousarray(np.asarray(inputs["bk"], dtype=np.float32))
    bva = np.ascontiguousarray(np.asarray(inputs["bv"], dtype=np.float32))
    xkT = [f16(k[b].T) for b in range(B)]
    xvT = [f16(v[b].T) for b in range(B)]
    in_maps = []
    for c in range(NCORES):
        b, qh = c // 2, c % 2
        in_maps.append({
            "xqT": f16(q[b, qh * SQ:(qh + 1) * SQ].T),
            "xkT": xkT[b], "xvT": xvT[b],
            "pos": tp[b],
            "posq": np.ascontiguousarray(tp[b, qh * SQ:(qh + 1) * SQ]),
            "wqT": wqT, "wkT": wkT, "wvT": wvT,
            "bq": bqa, "bk": bka, "bv": bva,
            "embT": embT,
        })
    return in_maps


def kernel(query, key, value, tile_positions, Wq, bq, Wk, bk, Wv, bv, emb_k):
    inputs = {"query": query, "key": key, "value": value,
              "tile_positions": tile_positions,
              "Wq": Wq, "bq": bq, "Wk": Wk, "bk": bk, "Wv": Wv, "bv": bv,
              "emb_k": emb_k}
    nc = _get_nc()
    in_maps = _make_in_maps(inputs)
    res = run_bass_kernel_spmd(nc, in_maps, core_ids=list(range(NCORES)))
    out = np.empty((B, S, D), np.float32)
    for c in range(NCORES):
        b, qh = c // 2, c % 2
        out[b, qh * SQ:(qh + 1) * SQ] = res.results[c]["out"]
    return out


# revision 7
# speedup vs baseline: 10.1722x; 1.1060x over previous
"""Distance-aware multihead attention on 8 Trainium2 NeuronCores (v3).

Problem: B=4, S=1024, D=768, H=12, DK=64, NUM_EMB=10.
  q/k/v = linear projections of query/key/value
  idx[b,i,j] = clip(round(9 * |pos_i - pos_j| / MAXD), 0, 9)
  logits = (q.k^T + qe[b,h,i,idx[b,i,j]]) / 8   where qe = q @ emb_k^T
  out = softmax(logits) @ v

Key decompositions:
  - bias qe[...,idx] = sum_{e=1..9} (qe_e - qe_{e-1}) * (d2 >= T_e^2); the
    qe_0 term is constant along the softmax axis and cancels.
  - step masks (d2 >= T_e^2) are shared across all 12 heads of a q-tile.
  - bias applied on the TENSOR engine: 9 diag(dqe_e) @ mask_e matmuls
    accumulated into the same PSUM group as the QK matmul (a diag matmul is
    a per-partition row scaling).  No per-element vector work for the bias.

v3: X and W arrive from the host already transposed ([in_dim, token] /
[in_dim, out_dim]) and cast to fp16, so DMA loads land in the exact SBUF
layout the PE needs - no on-chip transposition of inputs at all.  All PE
operands fp16 except P/V (bf16 for exp range).  Head loop is software-
pipelined (PE group for head h+1 issues before the transpose/AV tail of
head h).

Sharding: core c handles batch c//2, query-half c%2 (512 queries, all heads).
"""
import numpy as np

import concourse.bass as bass
import concourse.tile as tile
from concourse import bacc, mybir
from concourse.bass_utils import run_bass_kernel_spmd

F32 = mybir.dt.float32
BF16 = mybir.dt.bfloat16
FP16 = mybir.dt.float16
ACT = mybir.ActivationFunctionType
ALU = mybir.AluOpType

B, S, D = 4, 1024, 768
H, DK = 12, 64
NUM_EMB = 10
MAX_DIST = 100000.0 * 2 ** 0.5
SQ = S // 2          # queries per core
NQT = SQ // 128      # q-tiles per core (4)
NKT = S // 128       # k token chunks (8)
NDT = D // 128       # dim tiles (6)
NCORES = 8

# squared thresholds: idx >= e  <=>  d2 >= ((e-0.5)*MAX_DIST/9)^2
THRESH2 = [float(((e - 0.5) * MAX_DIST / 9.0) ** 2) for e in range(1, NUM_EMB)]


def build_nc():
    nc = bacc.Bacc("TRN2", target_bir_lowering=False, debug=False)

    # host-pretransposed fp16 operands: [contraction_dim, free_dim]
    xqT_d = nc.dram_tensor("xqT", [D, SQ], FP16, kind="ExternalInput").ap()
    xkT_d = nc.dram_tensor("xkT", [D, S], FP16, kind="ExternalInput").ap()
    xvT_d = nc.dram_tensor("xvT", [D, S], FP16, kind="ExternalInput").ap()
    wqT_d = nc.dram_tensor("wqT", [D, D], FP16, kind="ExternalInput").ap()
    wkT_d = nc.dram_tensor("wkT", [D, D], FP16, kind="ExternalInput").ap()
    wvT_d = nc.dram_tensor("wvT", [D, D], FP16, kind="ExternalInput").ap()
    embT_d = nc.dram_tensor("embT", [DK, NUM_EMB], FP16, kind="ExternalInput").ap()
    pos = nc.dram_tensor("pos", [S, 2], F32, kind="ExternalInput").ap()
    posq = nc.dram_tensor("posq", [SQ, 2], F32, kind="ExternalInput").ap()
    bq = nc.dram_tensor("bq", [D], F32, kind="ExternalInput").ap()
    bk = nc.dram_tensor("bk", [D], F32, kind="ExternalInput").ap()
    bv = nc.dram_tensor("bv", [D], F32, kind="ExternalInput").ap()
    out = nc.dram_tensor("out", [SQ, D], F32, kind="ExternalOutput").ap()

    with tile.TileContext(nc) as tc:
        with tc.tile_pool(name="persist", bufs=1) as persist:
            from concourse.masks import make_identity
            ident16 = persist.tile([128, 128], FP16)
            identb = persist.tile([128, 128], BF16)
            make_identity(nc, ident16[:])
            make_identity(nc, identb[:])

            # small loads on the Pool SWDGE queue
            bq_col = persist.tile([128, NDT], F32)
            bk_col = persist.tile([128, NDT], F32)
            nc.gpsimd.dma_start(out=bq_col[:], in_=bass.AP(tensor=bq.tensor, offset=0, ap=[[1, 128], [128, NDT]]))
            nc.gpsimd.dma_start(out=bk_col[:], in_=bass.AP(tensor=bk.tensor, offset=0, ap=[[1, 128], [128, NDT]]))
            bv_b = persist.tile([128, D], F32)
            nc.gpsimd.dma_start(out=bv_b[:], in_=bass.AP(tensor=bv.tensor, offset=0, ap=[[0, 128], [1, D]]))
            # position x/y as single-partition rows [1, S]
            posx_row = persist.tile([1, S], F32)
            posy_row = persist.tile([1, S], F32)
            nc.gpsimd.dma_start(out=posx_row[:], in_=bass.AP(tensor=pos.tensor, offset=0, ap=[[2, 1], [2, S]]))
            nc.gpsimd.dma_start(out=posy_row[:], in_=bass.AP(tensor=pos.tensor, offset=1, ap=[[2, 1], [2, S]]))
            # query positions as per-partition scalars [128, NQT]
            xq_col = persist.tile([128, NQT], F32)
            yq_col = persist.tile([128, NQT], F32)
            nc.gpsimd.dma_start(out=xq_col[:], in_=bass.AP(tensor=posq.tensor, offset=0, ap=[[2, 128], [256, NQT]]))
            nc.gpsimd.dma_start(out=yq_col[:], in_=bass.AP(tensor=posq.tensor, offset=1, ap=[[2, 128], [256, NQT]]))
            # emb^T block-diagonal [128, 2*NUM_EMB] fp16 (2 heads per matmul)
            embT_blk = persist.tile([128, 2 * NUM_EMB], FP16)
            nc.vector.memset(embT_blk[:], 0.0)
            nc.gpsimd.dma_start(out=embT_blk[0:64, 0:NUM_EMB], in_=embT_d[:, :])
            nc.gpsimd.dma_start(out=embT_blk[64:128, NUM_EMB:2 * NUM_EMB], in_=embT_d[:, :])

            ones1 = persist.tile([1, 128], F32)
            nc.vector.memset(ones1[:], 1.0)

            # persistent attention operands
            kT = persist.tile([128, NDT, S], FP16)      # K^T[dim, token] + bk
            qT = persist.tile([128, NDT, SQ], FP16)     # Q^T[dim, token] + bq
            v_sb = persist.tile([128, NKT, D], BF16)    # V[token, dim] (no bias)
            xk_b = persist.tile([128, S], F32)          # pos-x broadcast rows
            yk_b = persist.tile([128, S], F32)
            dqe = persist.tile([128, NQT, H, NUM_EMB - 1], F32)

            # ---- load (already transposed) + project ----
            with tc.tile_pool(name="tsp", bufs=2) as tsp, \
                 tc.tile_pool(name="pj_ps", bufs=2, space="PSUM") as pj_ps, \
                 tc.tile_pool(name="bc_ps", bufs=1, space="PSUM") as bc_ps, \
                 tc.tile_pool(name="qe_ps", bufs=1, space="PSUM") as qe_ps:

                # broadcast pos rows across partitions via 1-partition matmul
                for dst, row in ((xk_b, posx_row), (yk_b, posy_row)):
                    for hf in range(2):
                        sl = slice(512 * hf, 512 * hf + 512)
                        bc = bc_ps.tile([128, 512], F32, tag="bc")
                        nc.tensor.matmul(bc[:], ones1[:], row[:, sl],
                                         start=True, stop=True)
                        nc.scalar.copy(dst[:, sl], bc[:])

                def load_T(src, dst, ncols, dma_eng):
                    for d in range(NDT):
                        dma_eng.dma_start(out=dst[:, d, 0:ncols],
                                          in_=src[128 * d:128 * (d + 1), :])

                # V
                xvT = tsp.tile([128, NDT, S], FP16, tag="xT")
                wvT = tsp.tile([128, NDT, D], FP16, tag="wT")
                load_T(xvT_d, xvT, S, nc.sync)
                load_T(wvT_d, wvT, D, nc.scalar)
                for m in range(NKT):
                    for hf in range(2):
                        ps = pj_ps.tile([128, 512], F32, tag="pj")
                        for t in range(NDT):
                            nc.tensor.matmul(ps[:, 0:384], xvT[:, t, 128 * m:128 * m + 128],
                                             wvT[:, t, 384 * hf:384 * hf + 384],
                                             start=(t == 0), stop=(t == NDT - 1))
                        nc.scalar.copy(v_sb[:, m, 384 * hf:384 * hf + 384], ps[:, 0:384])

                # K
                xkT = tsp.tile([128, NDT, S], FP16, tag="xT")
                wkT = tsp.tile([128, NDT, D], FP16, tag="wT")
                load_T(xkT_d, xkT, S, nc.sync)
                load_T(wkT_d, wkT, D, nc.scalar)
                for m in range(NDT):
                    for hf in range(2):
                        ps = pj_ps.tile([128, 512], F32, tag="pj")
                        for t in range(NDT):
                            nc.tensor.matmul(ps[:], wkT[:, t, 128 * m:128 * m + 128],
                                             xkT[:, t, 512 * hf:512 * hf + 512],
                                             start=(t == 0), stop=(t == NDT - 1))
                        nc.scalar.activation(kT[:, m, 512 * hf:512 * hf + 512], ps[:],
                                             ACT.Identity, bias=bk_col[:, m:m + 1])

                # Q
                xqT = tsp.tile([128, NDT, S], FP16, tag="xT")
                wqT = tsp.tile([128, NDT, D], FP16, tag="wT")
                load_T(xqT_d, xqT, SQ, nc.sync)
                load_T(wqT_d, wqT, D, nc.scalar)
                for m in range(NDT):
                    ps = pj_ps.tile([128, 512], F32, tag="pj")
                    for t in range(NDT):
                        nc.tensor.matmul(ps[:], wqT[:, t, 128 * m:128 * m + 128],
                                         xqT[:, t, 0:SQ],
                                         start=(t == 0), stop=(t == NDT - 1))
                    nc.scalar.activation(qT[:, m, :], ps[:], ACT.Identity,
                                         bias=bq_col[:, m:m + 1])

                # qe for all q-tiles: block-diag emb matmul, 2 heads per 128-dim block
                qe_psum = qe_ps.tile([128, NQT, H * NUM_EMB], F32)
                for qt in range(NQT):
                    for m in range(NDT):
                        nc.tensor.matmul(qe_psum[:, qt, 20 * m:20 * m + 20],
                                         qT[:, m, 128 * qt:128 * qt + 128],
                                         embT_blk[:],
                                         start=True, stop=True)
                qe_sb = persist.tile([128, NQT, H, NUM_EMB], F32)
                nc.scalar.copy(qe_sb[:], qe_psum[:].rearrange("p q (h e) -> p q h e", e=NUM_EMB))
                nc.vector.tensor_tensor(out=dqe[:], in0=qe_sb[:, :, :, 1:],
                                        in1=qe_sb[:, :, :, :-1], op=ALU.subtract)

            # ---- attention ----
            with tc.tile_pool(name="att", bufs=2) as att, \
                 tc.tile_pool(name="prep", bufs=2) as prep, \
                 tc.tile_pool(name="qk_ps", bufs=2, space="PSUM") as qk_ps, \
                 tc.tile_pool(name="pt_ps", bufs=2, space="PSUM") as pt_ps, \
                 tc.tile_pool(name="av_ps", bufs=2, space="PSUM") as av_ps:

                def emit_prep(qt):
                    # d2 = |pos_k - pos_q|^2 for this q-tile, then 9 step masks
                    dx = prep.tile([128, S], F32, tag="dx")
                    dy = prep.tile([128, S], F32, tag="dy")
                    nc.vector.tensor_scalar(out=dx[:], in0=xk_b[:], scalar1=xq_col[:, qt:qt + 1],
                                            scalar2=None, op0=ALU.subtract)
                    nc.vector.tensor_scalar(out=dy[:], in0=yk_b[:], scalar1=yq_col[:, qt:qt + 1],
                                            scalar2=None, op0=ALU.subtract)
                    dx2 = prep.tile([128, S], F32, tag="dx2")
                    dy2 = prep.tile([128, S], F32, tag="dy2")
                    nc.scalar.square(dx2[:], dx[:])
                    nc.scalar.square(dy2[:], dy[:])
                    d2 = prep.tile([128, S], F32, tag="d2")
                    nc.vector.tensor_add(d2[:], dx2[:], dy2[:])
                    masks = prep.tile([128, NUM_EMB - 1, S], FP16, tag="masks")
                    for e in range(NUM_EMB - 1):
                        nc.vector.tensor_scalar(out=masks[:, e, :], in0=d2[:],
                                                scalar1=THRESH2[e], scalar2=None,
                                                op0=ALU.is_ge)
                    return masks

                masks_by_qt = {0: emit_prep(0)}
                # DVE-routed heads: bias applied via a serial STT chain on the
                # Vector engine instead of PE diag matmuls (engine balancing).
                DVE_HEADS = (10, 11)

                def emit_qk(qt, h, stop):
                    off = (64 * h) % 128
                    qk = qk_ps.tile([128, S], F32, tag="qk")
                    for hf in range(2):
                        sl = slice(512 * hf, 512 * hf + 512)
                        nc.tensor.matmul(qk[:, sl],
                                         qT[off:off + 64, h // 2, 128 * qt:128 * qt + 128],
                                         kT[off:off + 64, h // 2, sl],
                                         start=True, stop=stop)
                    return qk

                def emit_exp(src):
                    # bufs=6: tails lag fronts by up to ~5 units; exp(h) must
                    # never block on a den/p buffer held by a lagging tail
                    # (ACT-queue deadlock: tail's pT-copy sits behind exp(h)).
                    p_sb = att.tile([128, S], BF16, tag="p", bufs=6)
                    den = att.tile([128, 1], F32, tag="den", bufs=6)
                    nc.scalar.activation(p_sb[:], src[:], ACT.Exp, scale=0.125,
                                         accum_out=den[:])
                    return p_sb, den

                def emit_front(qt, h, masks):
                    # DVE: 9 per-head diag(dqe) builds
                    diag = att.tile([128, NUM_EMB - 1, 128], FP16, tag="diag")
                    for e in range(NUM_EMB - 1):
                        nc.vector.tensor_scalar(out=diag[:, e, :], in0=ident16[:],
                                                scalar1=dqe[:, qt, h, e:e + 1],
                                                scalar2=None, op0=ALU.mult)
                    # PE: qk + bias into one PSUM group per 512-half
                    off = (64 * h) % 128
                    qk = qk_ps.tile([128, S], F32, tag="qk")
                    for hf in range(2):
                        sl = slice(512 * hf, 512 * hf + 512)
                        nc.tensor.matmul(qk[:, sl],
                                         qT[off:off + 64, h // 2, 128 * qt:128 * qt + 128],
                                         kT[off:off + 64, h // 2, sl],
                                         start=True, stop=False)
                        for e in range(NUM_EMB - 1):
                            nc.tensor.matmul(qk[:, sl], diag[:, e, :], masks[:, e, sl],
                                             start=False, stop=(e == NUM_EMB - 2))
                    return emit_exp(qk)

                def make_chain_ops(qt, h, masks):
                    """DVE-route front for head h: QK on PE, then 9 STT bias ops
                    on DVE (returned as thunks for interleaved emission), then exp.
                    Returns (ops, finish) where finish() emits the exp."""
                    qk = emit_qk(qt, h, stop=True)
                    accs = [att.tile([128, S], F32, tag=f"chain{i}", name=f"chain_{qt}_{h}_{i}")
                            for i in range(2)]
                    state = {"src": qk, "e": 0}

                    def op():
                        e = state["e"]
                        dst = accs[e % 2]
                        nc.vector.scalar_tensor_tensor(
                            out=dst[:], in0=masks[:, e, :], scalar=dqe[:, qt, h, e:e + 1],
                            in1=state["src"][:], op0=ALU.mult, op1=ALU.add)
                        state["src"] = dst
                        state["e"] = e + 1

                    def finish():
                        return emit_exp(state["src"])

                    return [op] * (NUM_EMB - 1), finish

                def emit_tail(qt, h, p_sb, den, o_parts):
                    # PE: transpose P to [k, q] chunks; ACT: evacuate
                    ptp = pt_ps.tile([128, NKT, 128], BF16, tag="ptp")
                    for c in range(NKT):
                        nc.tensor.transpose(ptp[:, c, :], p_sb[:, 128 * c:128 * c + 128],
                                            identb[:])
                    pT = att.tile([128, NKT, 128], BF16, tag="pT")
                    nc.scalar.copy(pT[:], ptp[:])
                    # PE: AV accumulate over k chunks
                    av = av_ps.tile([128, DK], F32, tag="av")
                    for c in range(NKT):
                        nc.tensor.matmul(av[:], pT[:, c, :], v_sb[:, c, 64 * h:64 * h + 64],
                                         start=(c == 0), stop=(c == NKT - 1))
                    # DVE: out_h = av/den + bv_h
                    recip = att.tile([128, 1], F32, tag="recip")
                    nc.vector.reciprocal(recip[:], den[:])
                    nc.vector.scalar_tensor_tensor(
                        out=o_parts[:, h, :], in0=av[:], scalar=recip[:],
                        in1=bv_b[:, 64 * h:64 * h + 64], op0=ALU.mult, op1=ALU.add)

                pe_heads = [h for h in range(H) if h not in DVE_HEADS]
                for qt in range(NQT):
                    o_parts = att.tile([128, H, DK], F32, tag="o", name=f"o_{qt}")
                    masks = masks_by_qt.pop(qt)
                    tail_q = []      # (h, p_sb, den)
                    chains = []      # [ops_list, finish, h]
                    for i, h in enumerate(pe_heads):
                        tail_q.append((h,) + emit_front(qt, h, masks))
                        if i < len(DVE_HEADS):
                            ops, fin = make_chain_ops(qt, DVE_HEADS[i], masks)
                            chains.append([list(ops), fin, DVE_HEADS[i]])
                            chains[-1][0].pop(0)()  # first STT frees the qk bank
                        else:
                            # drain 2 chain STT ops, alternating between chains
                            for _ in range(2):
                                for ch in chains:
                                    if ch[0]:
                                        ch[0].pop(0)()
                                        break
                            for ch in [c for c in chains if not c[0] and c[1]]:
                                tail_q.append((ch[2],) + ch[1]())
                                ch[1] = None
                            if tail_q:
                                th, tp, td = tail_q.pop(0)
                                emit_tail(qt, th, tp, td, o_parts)
                        if i == 5 and qt + 1 < NQT:
                            masks_by_qt[qt + 1] = emit_prep(qt + 1)
                    # flush remaining chain ops / finishes / tails
                    for ch in chains:
                        while ch[0]:
                            ch[0].pop(0)()
                        if ch[1]:
                            tail_q.append((ch[2],) + ch[1]())
                            ch[1] = None
                    for th, tp, td in tail_q:
                        emit_tail(qt, th, tp, td, o_parts)
                    nc.sync.dma_start(out=out[128 * qt:128 * qt + 128, :],
                                      in_=o_parts[:].rearrange("p h d -> p (h d)"))
    nc.compile()
    return nc


_NC_CACHE = {}


def _get_nc():
    if "nc" not in _NC_CACHE:
        _NC_CACHE["nc"] = build_nc()
    return _NC_CACHE["nc"]


def _make_in_maps(inputs):
    q = np.asarray(inputs["query"], dtype=np.float32)
    k = np.asarray(inputs["key"], dtype=np.float32)
    v = np.asarray(inputs["value"], dtype=np.float32)
    tp = np.ascontiguousarray(np.asarray(inputs["tile_positions"], dtype=np.float32))
    f16 = lambda a: np.ascontiguousarray(a.astype(np.float16))
    wqT = f16(np.asarray(inputs["Wq"], dtype=np.float32).T)
    wkT = f16(np.asarray(inputs["Wk"], dtype=np.float32).T)
    wvT = f16(np.asarray(inputs["Wv"], dtype=np.float32).T)
    embT = f16(np.asarray(inputs["emb_k"], dtype=np.float32).T)
    bqa = np.ascontiguousarray(np.asarray(inputs["bq"], dtype=np.float32))
    bka = np.ascontiguousarray(np.asarray(inputs["bk"], dtype=np.float32))
    bva = np.ascontiguousarray(np.asarray(inputs["bv"], dtype=np.float32))
    xkT = [f16(k[b].T) for b in range(B)]
    xvT = [f16(v[b].T) for b in range(B)]
    in_maps = []
    for c in range(NCORES):
        b, qh = c // 2, c % 2
        in_maps.append({
            "xqT": f16(q[b, qh * SQ:(qh + 1) * SQ].T),
            "xkT": xkT[b], "xvT": xvT[b],
            "pos": tp[b],
            "posq": np.ascontiguousarray(tp[b, qh * SQ:(qh + 1) * SQ]),
            "wqT": wqT, "wkT": wkT, "wvT": wvT,
            "bq": bqa, "bk": bka, "bv": bva,
            "embT": embT,
        })
    return in_maps


def kernel(query, key, value, tile_positions, Wq, bq, Wk, bk, Wv, bv, emb_k):
    inputs = {"query": query, "key": key, "value": value,
              "tile_positions": tile_positions,
              "Wq": Wq, "bq": bq, "Wk": Wk, "bk": bk, "Wv": Wv, "bv": bv,
              "emb_k": emb_k}
    nc = _get_nc()
    in_maps = _make_in_maps(inputs)
    res = run_bass_kernel_spmd(nc, in_maps, core_ids=list(range(NCORES)))
    out = np.empty((B, S, D), np.float32)
    for c in range(NCORES):
        b, qh = c // 2, c % 2
        out[b, qh * SQ:(qh + 1) * SQ] = res.results[c]["out"]
    return out


# revision 8
# speedup vs baseline: 10.3311x; 1.0156x over previous
"""Distance-aware multihead attention on 8 Trainium2 NeuronCores (v3).

Problem: B=4, S=1024, D=768, H=12, DK=64, NUM_EMB=10.
  q/k/v = linear projections of query/key/value
  idx[b,i,j] = clip(round(9 * |pos_i - pos_j| / MAXD), 0, 9)
  logits = (q.k^T + qe[b,h,i,idx[b,i,j]]) / 8   where qe = q @ emb_k^T
  out = softmax(logits) @ v

Key decompositions:
  - bias qe[...,idx] = sum_{e=1..9} (qe_e - qe_{e-1}) * (d2 >= T_e^2); the
    qe_0 term is constant along the softmax axis and cancels.
  - step masks (d2 >= T_e^2) are shared across all 12 heads of a q-tile.
  - bias applied on the TENSOR engine: 9 diag(dqe_e) @ mask_e matmuls
    accumulated into the same PSUM group as the QK matmul (a diag matmul is
    a per-partition row scaling).  No per-element vector work for the bias.

v3: X and W arrive from the host already transposed ([in_dim, token] /
[in_dim, out_dim]) and cast to fp16, so DMA loads land in the exact SBUF
layout the PE needs - no on-chip transposition of inputs at all.  All PE
operands fp16 except P/V (bf16 for exp range).  Head loop is software-
pipelined (PE group for head h+1 issues before the transpose/AV tail of
head h).

Sharding: core c handles batch c//2, query-half c%2 (512 queries, all heads).
"""
import numpy as np

import concourse.bass as bass
import concourse.tile as tile
from concourse import bacc, mybir
from concourse.bass_utils import run_bass_kernel_spmd

F32 = mybir.dt.float32
BF16 = mybir.dt.bfloat16
FP16 = mybir.dt.float16
ACT = mybir.ActivationFunctionType
ALU = mybir.AluOpType

B, S, D = 4, 1024, 768
H, DK = 12, 64
NUM_EMB = 10
MAX_DIST = 100000.0 * 2 ** 0.5
SQ = S // 2          # queries per core
NQT = SQ // 128      # q-tiles per core (4)
NKT = S // 128       # k token chunks (8)
NDT = D // 128       # dim tiles (6)
NCORES = 8

# squared thresholds: idx >= e  <=>  d2 >= ((e-0.5)*MAX_DIST/9)^2
THRESH2 = [float(((e - 0.5) * MAX_DIST / 9.0) ** 2) for e in range(1, NUM_EMB)]


def build_nc():
    nc = bacc.Bacc("TRN2", target_bir_lowering=False, debug=False)

    # host-pretransposed fp16 operands: [contraction_dim, free_dim]
    xqT_d = nc.dram_tensor("xqT", [D, SQ], FP16, kind="ExternalInput").ap()
    xkT_d = nc.dram_tensor("xkT", [D, S], FP16, kind="ExternalInput").ap()
    xvT_d = nc.dram_tensor("xvT", [D, S], FP16, kind="ExternalInput").ap()
    wqT_d = nc.dram_tensor("wqT", [D, D], FP16, kind="ExternalInput").ap()
    wkT_d = nc.dram_tensor("wkT", [D, D], FP16, kind="ExternalInput").ap()
    wvT_d = nc.dram_tensor("wvT", [D, D], FP16, kind="ExternalInput").ap()
    embT_d = nc.dram_tensor("embT", [DK, NUM_EMB], FP16, kind="ExternalInput").ap()
    pos = nc.dram_tensor("pos", [S, 2], F32, kind="ExternalInput").ap()
    posq = nc.dram_tensor("posq", [SQ, 2], F32, kind="ExternalInput").ap()
    bq = nc.dram_tensor("bq", [D], F32, kind="ExternalInput").ap()
    bk = nc.dram_tensor("bk", [D], F32, kind="ExternalInput").ap()
    bv = nc.dram_tensor("bv", [D], F32, kind="ExternalInput").ap()
    out = nc.dram_tensor("out", [SQ, D], F32, kind="ExternalOutput").ap()

    with tile.TileContext(nc) as tc:
        with tc.tile_pool(name="persist", bufs=1) as persist:
            from concourse.masks import make_identity
            ident16 = persist.tile([128, 128], FP16)
            identb = persist.tile([128, 128], BF16)
            make_identity(nc, ident16[:])
            make_identity(nc, identb[:])

            # small loads on the Pool SWDGE queue
            bq_col = persist.tile([128, NDT], F32)
            bk_col = persist.tile([128, NDT], F32)
            nc.gpsimd.dma_start(out=bq_col[:], in_=bass.AP(tensor=bq.tensor, offset=0, ap=[[1, 128], [128, NDT]]))
            nc.gpsimd.dma_start(out=bk_col[:], in_=bass.AP(tensor=bk.tensor, offset=0, ap=[[1, 128], [128, NDT]]))
            bv_b = persist.tile([128, D], F32)
            nc.gpsimd.dma_start(out=bv_b[:], in_=bass.AP(tensor=bv.tensor, offset=0, ap=[[0, 128], [1, D]]))
            # position x/y as single-partition rows [1, S]
            posx_row = persist.tile([1, S], F32)
            posy_row = persist.tile([1, S], F32)
            nc.gpsimd.dma_start(out=posx_row[:], in_=bass.AP(tensor=pos.tensor, offset=0, ap=[[2, 1], [2, S]]))
            nc.gpsimd.dma_start(out=posy_row[:], in_=bass.AP(tensor=pos.tensor, offset=1, ap=[[2, 1], [2, S]]))
            # query positions as per-partition scalars [128, NQT]
            xq_col = persist.tile([128, NQT], F32)
            yq_col = persist.tile([128, NQT], F32)
            nc.gpsimd.dma_start(out=xq_col[:], in_=bass.AP(tensor=posq.tensor, offset=0, ap=[[2, 128], [256, NQT]]))
            nc.gpsimd.dma_start(out=yq_col[:], in_=bass.AP(tensor=posq.tensor, offset=1, ap=[[2, 128], [256, NQT]]))
            # emb^T block-diagonal [128, 2*NUM_EMB] fp16 (2 heads per matmul)
            embT_blk = persist.tile([128, 2 * NUM_EMB], FP16)
            nc.vector.memset(embT_blk[:], 0.0)
            nc.gpsimd.dma_start(out=embT_blk[0:64, 0:NUM_EMB], in_=embT_d[:, :])
            nc.gpsimd.dma_start(out=embT_blk[64:128, NUM_EMB:2 * NUM_EMB], in_=embT_d[:, :])

            ones1 = persist.tile([1, 128], F32)
            nc.vector.memset(ones1[:], 1.0)

            # persistent attention operands
            kT = persist.tile([128, NDT, S], FP16)      # K^T[dim, token] + bk
            qT = persist.tile([128, NDT, SQ], FP16)     # Q^T[dim, token] + bq
            v_sb = persist.tile([128, NKT, D], BF16)    # V[token, dim] (no bias)
            xk_b = persist.tile([128, S], F32)          # pos-x broadcast rows
            yk_b = persist.tile([128, S], F32)
            dqe = persist.tile([128, NQT, H, NUM_EMB - 1], F32)

            # ---- load (already transposed) + project ----
            with tc.tile_pool(name="tsp", bufs=2) as tsp, \
                 tc.tile_pool(name="pj_ps", bufs=2, space="PSUM") as pj_ps, \
                 tc.tile_pool(name="bc_ps", bufs=1, space="PSUM") as bc_ps, \
                 tc.tile_pool(name="qe_ps", bufs=1, space="PSUM") as qe_ps:

                # broadcast pos rows across partitions via 1-partition matmul
                for dst, row in ((xk_b, posx_row), (yk_b, posy_row)):
                    for hf in range(2):
                        sl = slice(512 * hf, 512 * hf + 512)
                        bc = bc_ps.tile([128, 512], F32, tag="bc")
                        nc.tensor.matmul(bc[:], ones1[:], row[:, sl],
                                         start=True, stop=True)
                        nc.scalar.copy(dst[:, sl], bc[:])

                def load_T(src, dst, ncols, dma_eng):
                    for d in range(NDT):
                        dma_eng.dma_start(out=dst[:, d, 0:ncols],
                                          in_=src[128 * d:128 * (d + 1), :])

                # V
                xvT = tsp.tile([128, NDT, S], FP16, tag="xT")
                wvT = tsp.tile([128, NDT, D], FP16, tag="wT")
                load_T(xvT_d, xvT, S, nc.sync)
                load_T(wvT_d, wvT, D, nc.scalar)
                for m in range(NKT):
                    for hf in range(2):
                        ps = pj_ps.tile([128, 512], F32, tag="pj")
                        for t in range(NDT):
                            nc.tensor.matmul(ps[:, 0:384], xvT[:, t, 128 * m:128 * m + 128],
                                             wvT[:, t, 384 * hf:384 * hf + 384],
                                             start=(t == 0), stop=(t == NDT - 1))
                        nc.scalar.copy(v_sb[:, m, 384 * hf:384 * hf + 384], ps[:, 0:384])

                # K
                xkT = tsp.tile([128, NDT, S], FP16, tag="xT")
                wkT = tsp.tile([128, NDT, D], FP16, tag="wT")
                load_T(xkT_d, xkT, S, nc.sync)
                load_T(wkT_d, wkT, D, nc.scalar)
                for m in range(NDT):
                    for hf in range(2):
                        ps = pj_ps.tile([128, 512], F32, tag="pj")
                        for t in range(NDT):
                            nc.tensor.matmul(ps[:], wkT[:, t, 128 * m:128 * m + 128],
                                             xkT[:, t, 512 * hf:512 * hf + 512],
                                             start=(t == 0), stop=(t == NDT - 1))
                        nc.scalar.activation(kT[:, m, 512 * hf:512 * hf + 512], ps[:],
                                             ACT.Identity, bias=bk_col[:, m:m + 1])

                # Q
                xqT = tsp.tile([128, NDT, S], FP16, tag="xT")
                wqT = tsp.tile([128, NDT, D], FP16, tag="wT")
                load_T(xqT_d, xqT, SQ, nc.sync)
                load_T(wqT_d, wqT, D, nc.scalar)
                for m in range(NDT):
                    ps = pj_ps.tile([128, 512], F32, tag="pj")
                    for t in range(NDT):
                        nc.tensor.matmul(ps[:], wqT[:, t, 128 * m:128 * m + 128],
                                         xqT[:, t, 0:SQ],
                                         start=(t == 0), stop=(t == NDT - 1))
                    nc.scalar.activation(qT[:, m, :], ps[:], ACT.Identity,
                                         bias=bq_col[:, m:m + 1])

                # qe for all q-tiles: block-diag emb matmul, 2 heads per 128-dim block
                qe_psum = qe_ps.tile([128, NQT, H * NUM_EMB], F32)
                for qt in range(NQT):
                    for m in range(NDT):
                        nc.tensor.matmul(qe_psum[:, qt, 20 * m:20 * m + 20],
                                         qT[:, m, 128 * qt:128 * qt + 128],
                                         embT_blk[:],
                                         start=True, stop=True)
                qe_sb = persist.tile([128, NQT, H, NUM_EMB], F32)
                nc.scalar.copy(qe_sb[:], qe_psum[:].rearrange("p q (h e) -> p q h e", e=NUM_EMB))
                nc.vector.tensor_tensor(out=dqe[:], in0=qe_sb[:, :, :, 1:],
                                        in1=qe_sb[:, :, :, :-1], op=ALU.subtract)

            # ---- attention ----
            with tc.tile_pool(name="att", bufs=2) as att, \
                 tc.tile_pool(name="prep", bufs=2) as prep, \
                 tc.tile_pool(name="qk_ps", bufs=2, space="PSUM") as qk_ps, \
                 tc.tile_pool(name="pt_ps", bufs=2, space="PSUM") as pt_ps, \
                 tc.tile_pool(name="av_ps", bufs=2, space="PSUM") as av_ps:

                def emit_prep(qt):
                    # d2 = |pos_k - pos_q|^2 for this q-tile, then 9 step masks
                    dx = prep.tile([128, S], F32, tag="dx")
                    dy = prep.tile([128, S], F32, tag="dy")
                    nc.vector.tensor_scalar(out=dx[:], in0=xk_b[:], scalar1=xq_col[:, qt:qt + 1],
                                            scalar2=None, op0=ALU.subtract)
                    nc.vector.tensor_scalar(out=dy[:], in0=yk_b[:], scalar1=yq_col[:, qt:qt + 1],
                                            scalar2=None, op0=ALU.subtract)
                    dx2 = prep.tile([128, S], F32, tag="dx2")
                    dy2 = prep.tile([128, S], F32, tag="dy2")
                    nc.scalar.square(dx2[:], dx[:])
                    nc.scalar.square(dy2[:], dy[:])
                    d2 = prep.tile([128, S], F32, tag="d2")
                    nc.vector.tensor_add(d2[:], dx2[:], dy2[:])
                    masks = prep.tile([128, NUM_EMB - 1, S], FP16, tag="masks")
                    for e in range(NUM_EMB - 1):
                        nc.vector.tensor_scalar(out=masks[:, e, :], in0=d2[:],
                                                scalar1=THRESH2[e], scalar2=None,
                                                op0=ALU.is_ge)
                    return masks

                masks_by_qt = {0: emit_prep(0)}
                # DVE-routed heads: bias applied via a serial STT chain on the
                # Vector engine instead of PE diag matmuls (engine balancing).
                DVE_HEADS = (9, 10, 11)

                def emit_qk(qt, h, stop):
                    off = (64 * h) % 128
                    qk = qk_ps.tile([128, S], F32, tag="qk")
                    for hf in range(2):
                        sl = slice(512 * hf, 512 * hf + 512)
                        nc.tensor.matmul(qk[:, sl],
                                         qT[off:off + 64, h // 2, 128 * qt:128 * qt + 128],
                                         kT[off:off + 64, h // 2, sl],
                                         start=True, stop=stop)
                    return qk

                def emit_exp(src):
                    # bufs=6: tails lag fronts by up to ~5 units; exp(h) must
                    # never block on a den/p buffer held by a lagging tail
                    # (ACT-queue deadlock: tail's pT-copy sits behind exp(h)).
                    p_sb = att.tile([128, S], BF16, tag="p", bufs=8)
                    den = att.tile([128, 1], F32, tag="den", bufs=8)
                    nc.scalar.activation(p_sb[:], src[:], ACT.Exp, scale=0.125,
                                         accum_out=den[:])
                    return p_sb, den

                def emit_front(qt, h, masks):
                    # DVE: 9 per-head diag(dqe) builds
                    diag = att.tile([128, NUM_EMB - 1, 128], FP16, tag="diag")
                    for e in range(NUM_EMB - 1):
                        nc.vector.tensor_scalar(out=diag[:, e, :], in0=ident16[:],
                                                scalar1=dqe[:, qt, h, e:e + 1],
                                                scalar2=None, op0=ALU.mult)
                    # PE: qk + bias into one PSUM group per 512-half
                    off = (64 * h) % 128
                    qk = qk_ps.tile([128, S], F32, tag="qk")
                    for hf in range(2):
                        sl = slice(512 * hf, 512 * hf + 512)
                        nc.tensor.matmul(qk[:, sl],
                                         qT[off:off + 64, h // 2, 128 * qt:128 * qt + 128],
                                         kT[off:off + 64, h // 2, sl],
                                         start=True, stop=False)
                        for e in range(NUM_EMB - 1):
                            nc.tensor.matmul(qk[:, sl], diag[:, e, :], masks[:, e, sl],
                                             start=False, stop=(e == NUM_EMB - 2))
                    return emit_exp(qk)

                def make_chain_ops(qt, h, masks):
                    """DVE-route front for head h: QK on PE, then 9 STT bias ops
                    on DVE (returned as thunks for interleaved emission), then exp.
                    Returns (ops, finish) where finish() emits the exp."""
                    qk = emit_qk(qt, h, stop=True)
                    accs = [att.tile([128, S], F32, tag=f"chain{i}", name=f"chain_{qt}_{h}_{i}")
                            for i in range(2)]
                    state = {"src": qk, "e": 0}

                    def op():
                        e = state["e"]
                        dst = accs[e % 2]
                        nc.vector.scalar_tensor_tensor(
                            out=dst[:], in0=masks[:, e, :], scalar=dqe[:, qt, h, e:e + 1],
                            in1=state["src"][:], op0=ALU.mult, op1=ALU.add)
                        state["src"] = dst
                        state["e"] = e + 1

                    def finish():
                        return emit_exp(state["src"])

                    return [op] * (NUM_EMB - 1), finish

                def emit_tail(qt, h, p_sb, den, o_parts):
                    # PE: transpose P to [k, q] chunks; ACT: evacuate
                    ptp = pt_ps.tile([128, NKT, 128], BF16, tag="ptp")
                    for c in range(NKT):
                        nc.tensor.transpose(ptp[:, c, :], p_sb[:, 128 * c:128 * c + 128],
                                            identb[:])
                    pT = att.tile([128, NKT, 128], BF16, tag="pT")
                    nc.scalar.copy(pT[:], ptp[:])
                    # PE: AV accumulate over k chunks
                    av = av_ps.tile([128, DK], F32, tag="av")
                    for c in range(NKT):
                        nc.tensor.matmul(av[:], pT[:, c, :], v_sb[:, c, 64 * h:64 * h + 64],
                                         start=(c == 0), stop=(c == NKT - 1))
                    # DVE: out_h = av/den + bv_h
                    recip = att.tile([128, 1], F32, tag="recip")
                    nc.vector.reciprocal(recip[:], den[:])
                    nc.vector.scalar_tensor_tensor(
                        out=o_parts[:, h, :], in0=av[:], scalar=recip[:],
                        in1=bv_b[:, 64 * h:64 * h + 64], op0=ALU.mult, op1=ALU.add)

                pe_heads = [h for h in range(H) if h not in DVE_HEADS]
                for qt in range(NQT):
                    o_parts = att.tile([128, H, DK], F32, tag="o", name=f"o_{qt}")
                    masks = masks_by_qt.pop(qt)
                    tail_q = []      # (h, p_sb, den)
                    chains = []      # [ops_list, finish, h]
                    for i, h in enumerate(pe_heads):
                        tail_q.append((h,) + emit_front(qt, h, masks))
                        if i < len(DVE_HEADS):
                            ops, fin = make_chain_ops(qt, DVE_HEADS[i], masks)
                            chains.append([list(ops), fin, DVE_HEADS[i]])
                            chains[-1][0].pop(0)()  # first STT frees the qk bank
                        else:
                            # drain 4 chain STT ops, alternating between chains
                            for _ in range(4):
                                for ch in chains:
                                    if ch[0]:
                                        ch[0].pop(0)()
                                        break
                            for ch in [c for c in chains if not c[0] and c[1]]:
                                tail_q.append((ch[2],) + ch[1]())
                                ch[1] = None
                            if tail_q:
                                th, tp, td = tail_q.pop(0)
                                emit_tail(qt, th, tp, td, o_parts)
                        if i == 5 and qt + 1 < NQT:
                            masks_by_qt[qt + 1] = emit_prep(qt + 1)
                    # flush remaining chain ops / finishes / tails
                    for ch in chains:
                        while ch[0]:
                            ch[0].pop(0)()
                        if ch[1]:
                            tail_q.append((ch[2],) + ch[1]())
                            ch[1] = None
                    for th, tp, td in tail_q:
                        emit_tail(qt, th, tp, td, o_parts)
                    nc.sync.dma_start(out=out[128 * qt:128 * qt + 128, :],
                                      in_=o_parts[:].rearrange("p h d -> p (h d)"))
    nc.compile()
    return nc


_NC_CACHE = {}


def _get_nc():
    if "nc" not in _NC_CACHE:
        _NC_CACHE["nc"] = build_nc()
    return _NC_CACHE["nc"]


def _make_in_maps(inputs):
    q = np.asarray(inputs["query"], dtype=np.float32)
    k = np.asarray(inputs["key"], dtype=np.float32)
    v = np.asarray(inputs["value"], dtype=np.float32)
    tp = np.ascontiguousarray(np.asarray(inputs["tile_positions"], dtype=np.float32))
    f16 = lambda a: np.ascontiguousarray(a.astype(np.float16))
    wqT = f16(np.asarray(inputs["Wq"], dtype=np.float32).T)
    wkT = f16(np.asarray(inputs["Wk"], dtype=np.float32).T)
    wvT = f16(np.asarray(inputs["Wv"], dtype=np.float32).T)
    embT = f16(np.asarray(inputs["emb_k"], dtype=np.float32).T)
    bqa = np.ascontiguousarray(np.asarray(inputs["bq"], dtype=np.float32))
    bka = np.ascontiguousarray(np.asarray(inputs["bk"], dtype=np.float32))
    bva = np.ascontiguousarray(np.asarray(inputs["bv"], dtype=np.float32))
    xkT = [f16(k[b].T) for b in range(B)]
    xvT = [f16(v[b].T) for b in range(B)]
    in_maps = []
    for c in range(NCORES):
        b, qh = c // 2, c % 2
        in_maps.append({
            "xqT": f16(q[b, qh * SQ:(qh + 1) * SQ].T),
            "xkT": xkT[b], "xvT": xvT[b],
            "pos": tp[b],
            "posq": np.ascontiguousarray(tp[b, qh * SQ:(qh + 1) * SQ]),
            "wqT": wqT, "wkT": wkT, "wvT": wvT,
            "bq": bqa, "bk": bka, "bv": bva,
            "embT": embT,
        })
    return in_maps


def kernel(query, key, value, tile_positions, Wq, bq, Wk, bk, Wv, bv, emb_k):
    inputs = {"query": query, "key": key, "value": value,
              "tile_positions": tile_positions,
              "Wq": Wq, "bq": bq, "Wk": Wk, "bk": bk, "Wv": Wv, "bv": bv,
              "emb_k": emb_k}
    nc = _get_nc()
    in_maps = _make_in_maps(inputs)
    res = run_bass_kernel_spmd(nc, in_maps, core_ids=list(range(NCORES)))
    out = np.empty((B, S, D), np.float32)
    for c in range(NCORES):
        b, qh = c // 2, c % 2
        out[b, qh * SQ:(qh + 1) * SQ] = res.results[c]["out"]
    return out
